# revision 102
# baseline (speedup 1.0000x reference)
"""Trainium2 Bass kernel for nn_AttentionModelCharLevel.

Model (per reference): visual linear -> char-encoder LSTM -> linear+relu ->
cosine attention (softmax over batch dim) -> char-decoder LSTM -> per-sample
mean NLL over L steps.

Sharding: data-parallel over batch B=4096 across 8 cores (512 rows each).
The [B,B] attention needs every core to see all normalized visual rows, so
each core computes its vn shard (+transpose), AllGathers both, and streams
the gathered rows back through SBUF during the attention phase.

Key device-side conventions (v2, fp8 DoubleRow):
- The LSTM recurrence, encoder linear and decoder logits matmuls run in
  fp8e4 with perf_mode=DoubleRow (2 fp8 K-rows per PE cell): K=512
  contractions become 2 matmuls of logical K=256. Weights are pre-scaled
  x64 host-side (fp8 normal range) and the x1/64 is folded into the ACT
  scale that reads the PSUM.
- Hidden state is stored doubled (Ht = 2h) as a single [128, 4, B] fp8
  tile per step; slice [:, 2g:2g+2, :] is the DoubleRow rhs pair. Cell
  state St = 2c is a [128, 4, B] bf16 tile.
- sigmoid(z) = 0.5*tanh(z/2)+0.5; the g-gate's weights carry an extra x2
  so ALL gates share one ACT scale (0.5/64). The bias is folded into the
  x-part matmul as a 51st embedding row (x=1), so one ACT with no bias
  covers a whole 4-bank PSUM quad [128, 4, B] = the 4 gates of one hidden
  chunk -> 4 gate ACTs + 1 tanh(c) ACT per step instead of 20.
- The DVE tail chain per hidden chunk runs in bf16 (2x DVE rate):
      m2 = (tanh_i + 1) * tanh_g
      m1 = (tanh_f + 1) * S
      S' = 0.5*m1 + m2
      H' = (tanh_o + 1) * tanh(0.5*S')   (fp8 out)
- Softmax over the batch dim reduces to exp() only: sims are cosine
  similarities in [-1,1] and the softmax denominator is a positive
  per-column scale that the final row normalization of h cancels.
- Decoder log-softmax: logits kept in [V, B] layout; Z = ones^T exp(logits)
  and the target logit via a one-hot mask built from an iota column --
  both reduce over partitions with K=128 ones-matmuls into spare banks of
  the rotating PSUM quads. ln() deferred to a single post-pass.
"""
import os
import sys

sys.path.insert(0, '/opt/trn_rl_repo')

import numpy as np

B_FULL = 4096
NCORES = 8
B = B_FULL // NCORES          # 512 rows per core
H = 512
G = 4 * H                     # 2048
E = 50
EA = E + 1                    # embedding dim + bias row
L = 16
V = 128
VIS = 2048
HK = H // 128                 # 4 chunks of the hidden dim
GK = G // 128                 # 16 gate chunks
VISK = VIS // 128             # 16 chunks of the visual dim
BK = B // 128                 # 4 batch chunks per core
VCHUNKS = B_FULL // 128       # 32 chunks of the full batch
SF = 64.0                     # fp8 weight scale

_CACHE = {}


def _build():
    import concourse.bass as bass
    import concourse.tile as tile
    import concourse.mybir as mybir
    from concourse import bacc
    from concourse.masks import make_identity
    from contextlib import ExitStack

    dt = mybir.dt
    AF = mybir.ActivationFunctionType
    ALU = mybir.AluOpType
    DR = mybir.MatmulPerfMode.DoubleRow
    f32 = dt.float32
    f32r = dt.float32r
    bf16d = dt.bfloat16
    f8 = dt.float8e4
    LN2 = float(np.log(2.0))

    AP = bass.AP
    nc = bacc.Bacc("TRN2", target_bir_lowering=False, debug=False,
                   num_devices=NCORES)

    # ---- DRAM I/O ----
    visT_d = nc.dram_tensor("visT", [VISK // 2, 128, 2, B], f8, kind="ExternalInput").ap()
    WvisT_d = nc.dram_tensor("WvisT", [VISK // 2, 128, 2, H], f8, kind="ExternalInput").ap()
    WihT_d = nc.dram_tensor("WihT", [EA, G], bf16d, kind="ExternalInput").ap()
    Whh8_d = nc.dram_tensor("Whh8", [128, HK, G], f8, kind="ExternalInput").ap()
    encx_d = nc.dram_tensor("encx", [L, EA, B], bf16d, kind="ExternalInput").ap()
    decx_d = nc.dram_tensor("decx", [L, EA, B], bf16d, kind="ExternalInput").ap()
    Wenc8_d = nc.dram_tensor("Wenc8", [128, HK, H], f8, kind="ExternalInput").ap()
    benc_d = nc.dram_tensor("benc", [128, HK], f32, kind="ExternalInput").ap()
    Wout8_d = nc.dram_tensor("Wout8", [128, HK, V], f8, kind="ExternalInput").ap()
    tgt_d = nc.dram_tensor("tgt", [L, B], f32, kind="ExternalInput").ap()
    iota_d = nc.dram_tensor("iota128", [128, 1], f32, kind="ExternalInput").ap()
    oneh_d = nc.dram_tensor("oneh", [L, 128, B], f8, kind="ExternalInput").ap()
    ones_d = nc.dram_tensor("ones128", [128, 1], f32r, kind="ExternalInput").ap()
    out_d = nc.dram_tensor("loss", [1, B], f32, kind="ExternalOutput").ap()

    with tile.TileContext(nc) as tc, ExitStack() as top:
        wpool = top.enter_context(tc.tile_pool(name="w", bufs=1))
        spool = top.enter_context(tc.tile_pool(name="state", bufs=2))
        dram = top.enter_context(tc.tile_pool(name="dram", bufs=1, space="DRAM"))

        # ---- persistent weights / constants (DMAs issued after the visual
        # input DMAs: the scan weights aren't needed until the encoder) ----
        Whh8 = wpool.tile([128, HK, G], f8, tag="whh8", name="whh8")
        WihT2 = wpool.tile([64 + EA, G], bf16d, tag="wih", name="wih")
        Wenc8 = wpool.tile([128, HK, H], f8, tag="wenc8", name="wenc8")
        benc = wpool.tile([128, HK], f32, tag="benc", name="benc")
        Wout8 = wpool.tile([128, HK, V], f8, tag="wout8", name="wout8")

        def load_weights():
            nc.sync.dma_start(Whh8[:], Whh8_d)
            nc.sync.dma_start(WihT2[:EA, :], WihT_d)
            nc.sync.dma_start(WihT2[64:64 + EA, :], WihT_d)
            nc.sync.dma_start(Wenc8[:], Wenc8_d)
            nc.sync.dma_start(benc[:], benc_d)
            nc.sync.dma_start(Wout8[:], Wout8_d)

        ones_col = wpool.tile([128, 1], f32r, tag="ones_col", name="ones_col")
        nc.sync.dma_start(ones_col[:], ones_d)
        ones_row = wpool.tile([1, 128], f32r, tag="ones_row", name="ones_row")
        nc.sync.dma_start(ones_row[:], ones_d.rearrange("p one -> one p"))
        ones16 = wpool.tile([L, 1], f32r, tag="ones16", name="ones16")
        nc.sync.dma_start(ones16[:], ones_d[:L])
        ident8 = wpool.tile([128, 128], f8, tag="ident8", name="ident8")
        make_identity(nc, ident8[:])

        def warm_pe(out_ap, n):
            # dummy matmuls into a region the next real start=True matmul
            # resets; they run in otherwise-idle PE windows and keep the
            # HAM activity window busy so real matmuls stay at 2.4 GHz
            for _ in range(n):
                nc.tensor.matmul(out_ap, ident8[:], ident8[:],
                                 start=True, stop=True,
                                 skip_group_check=True)
        ln2_t = wpool.tile([1, 1], f32, tag="ln2", name="ln2")
        nc.vector.memset(ln2_t[:], LN2)
        ln16_t = wpool.tile([1, 1], f32, tag="ln16", name="ln16")
        nc.vector.memset(ln16_t[:], float(np.log(16.0)))
        ln16c = wpool.tile([128, 1], f32, tag="ln16c", name="ln16c")
        nc.vector.memset(ln16c[:], float(np.log(16.0)))

        # AllGather buffers: vn blocks and vnT blocks (both fp8, x16 scale)
        ag_in = dram.tile([BK, 128, B], f8, name="ag_in")
        ag_out = dram.tile([NCORES, BK, 128, B], f8, addr_space="Shared", name="ag_out")
        agt_in = dram.tile([HK, 128, B], f8, name="agt_in")
        agt_out = dram.tile([NCORES, HK, 128, B], f8, addr_space="Shared", name="agt_out")

        # decoder per-step Z and target-logit rows
        zpool = top.enter_context(tc.tile_pool(name="zp", bufs=1))
        Zboth = zpool.tile([L, 2, B], f32r, tag="Zb", name="Zb")
        Zs = Zboth[:, 0, :]
        lts = Zboth[:, 1, :]

        # visual inputs land first, split across BOTH DMA queues so the
        # visual matmuls aren't paced by one queue's dispatch rate; the
        # pool closes after phase 1 to return the SBUF (LIFO: innermost)
        vload_stack = ExitStack()
        vload = vload_stack.enter_context(tc.tile_pool(name="vload", bufs=1))
        vis_ts = [vload.tile([128, 2, B], f8, tag=f"vis{k}", name=f"vis{k}")
                  for k in range(VISK // 2)]
        wv_ts = [vload.tile([128, 2, H], f8, tag=f"wvis{k}", name=f"wvis{k}")
                 for k in range(VISK // 2)]
        for ki in range(VISK // 2):
            nc.gpsimd.dma_start(vis_ts[ki][:], visT_d[ki])
            nc.sync.dma_start(wv_ts[ki][:], WvisT_d[ki])

        # prefetch ALL scan inputs up front on the gpsimd DMA queue so the
        # LSTM scans never wait on the sync queue / collective window
        xenc = wpool.tile([64 + EA, L, B], bf16d, tag="xenc", name="xenc")
        xdec = wpool.tile([64 + EA, L, B], bf16d, tag="xdec", name="xdec")
        for s in range(L):
            nc.gpsimd.dma_start(xenc[:EA, s, :], encx_d[s])
            nc.gpsimd.dma_start(xenc[64:64 + EA, s, :], encx_d[s])
        for s in range(L):
            nc.gpsimd.dma_start(xdec[:EA, s, :], decx_d[s])
            nc.gpsimd.dma_start(xdec[64:64 + EA, s, :], decx_d[s])
        ohall = wpool.tile([128, L, B], f8, tag="ohall", name="ohall")
        nc.gpsimd.dma_start(ohall[:], oneh_d.rearrange("l p b -> p l b"))

        # ======== Phase 1: visual linear + row-normalize + transpose + AG ====
        with ExitStack() as ph:
            vsb = ph.enter_context(tc.tile_pool(name="vsb", bufs=3))
            vps = ph.enter_context(tc.tile_pool(name="vps", bufs=1, space="PSUM"))
            tps = ph.enter_context(tc.tile_pool(name="tps", bufs=4, space="PSUM"))
            vnpool = ph.enter_context(tc.tile_pool(name="vnp", bufs=1))

            v_ps = [vps.tile([128, H], f32, tag=f"vps{b}", name=f"vps{b}") for b in range(BK)]
            for ki in range(VISK // 2):
                vis_t, wv_t = vis_ts[ki], wv_ts[ki]
                for b in range(BK):
                    nc.tensor.matmul(v_ps[b][:], vis_t[:, :, b * 128:(b + 1) * 128],
                                     wv_t[:], start=(ki == 0),
                                     stop=(ki == VISK // 2 - 1), perf_mode=DR)
            load_weights()
            # batched by function so the ACT table isn't thrashed
            s_cols, vn = [], []
            for b in range(BK):
                sq = vsb.tile([128, H], f32, tag="vsq", name="vsq")
                s_col = vsb.tile([128, 1], f32, tag=f"vscol{b}", name=f"vscol{b}", bufs=1)
                nc.scalar.activation(sq[:], v_ps[b][:], AF.Square,
                                     accum_out=s_col[:])
                s_cols.append(s_col)
            lns = []
            for b in range(BK):
                lnc_ = vsb.tile([128, 1], f32, tag=f"vln{b}", name=f"vln{b}", bufs=1)
                nc.scalar.activation(lnc_[:], s_cols[b][:], AF.Ln)
                lns.append(lnc_)
            # sqrt(s) = 64*|v|, so exp(-0.5*ln s + ln16) puts vn in fp8 x16
            rss = []
            for b in range(BK):
                rs = vsb.tile([128, 1], f32, tag=f"vrs{b}", name=f"vrs{b}", bufs=1)
                nc.scalar.activation(rs[:], lns[b][:], AF.Exp, scale=-0.5,
                                     bias=ln16c[:])
                rss.append(rs)
            for b in range(BK):
                vn_b = vnpool.tile([128, H], f8, tag=f"vn{b}", name=f"vn{b}")
                nc.vector.tensor_scalar(vn_b[:], v_ps[b][:], rss[b][:], None, ALU.mult)
                vn.append(vn_b)
            # transpose vn -> vnT (16 128x128 blocks, fp8 for the sims lhsT)
            vnT = [vnpool.tile([128, B], f8, tag=f"vnT{h}", name=f"vnT{h}") for h in range(HK)]
            for b in range(BK):
                for h in range(HK):
                    t_ps = tps.tile([128, 128, 2], f8, tag="tr", name="tr")
                    nc.tensor.transpose(
                        t_ps[:, :, 0], vn[b][:, h * 128:(h + 1) * 128], ident8[:])
                    nc.vector.tensor_copy(vnT[h][:, b * 128:(b + 1) * 128],
                                          t_ps[:, :, 0])
            for b in range(BK):
                nc.sync.dma_start(ag_in[b], vn[b][:])
            for h in range(HK):
                nc.sync.dma_start(agt_in[h], vnT[h][:])
            nc.gpsimd.collective_compute(
                "AllGather", mybir.AluOpType.bypass,
                replica_groups=[list(range(NCORES))],
                ins=[ag_in[:]], outs=[ag_out[:]],
            )
            nc.gpsimd.collective_compute(
                "AllGather", mybir.AluOpType.bypass,
                replica_groups=[list(range(NCORES))],
                ins=[agt_in[:]], outs=[agt_out[:]],
            )

        vload_stack.close()
        # staged full vnT (fp8, DoubleRow pair layout) for the sims lhsT;
        # filled by gpsimd DMAs that wait on the collective during the encoder
        vnTp = [wpool.tile([128, 2, B_FULL], f8, tag=f"vnTp{g}", name=f"vnTp{g}")
                for g in range(2)]
        for k in range(HK):
            for r in range(NCORES):
                nc.gpsimd.dma_start(vnTp[k // 2][:, k % 2, r * B:(r + 1) * B],
                                    agt_out[r, k])

        # ======== LSTM scan helper ========
        gsb = top.enter_context(tc.tile_pool(name="gsb", bufs=3))
        msb = top.enter_context(tc.tile_pool(name="msb", bufs=4))

        from collections import deque

        def lstm_step(gps, xall, s, Hp8, Sp, max_open=2):
            """One LSTM step, fp8 DoubleRow recurrence, quad-bank PSUM.

            Per hidden chunk j one 4-bank PSUM quad holds gates i,f,g,o.
            open = x-part matmuls (K=51, two concurrent row groups) plus the
            h01 DoubleRow pair; close = h23 pair + one mega-ACT over the quad
            + the state chain split across gpsimd (m2, m1) and DVE (S', H'),
            with tanh(c) in chunk pairs on ACT. Closes lag opens so the PE
            streams the next chunk while the previous drains, and the first
            closed chunks feed the next step's opens.
            """
            Hn8 = spool.tile([128, HK, B], f8, tag="H", name="H")
            Sn = spool.tile([128, HK, B], bf16d, tag="S", name="S")
            Ts = {}
            pairs = {}

            def xmms(j, half):
                pt = gps.tile([128, 2, B], f32, tag="pair", name="pair")
                pairs[(j, half)] = pt
                for gi in range(2):
                    gate = half * 2 + gi
                    c = gate * 4 + j
                    r0 = 0 if gi == 0 else 64
                    nc.tensor.matmul(pt[:, gi, :],
                                     WihT2[r0:r0 + EA, c * 128:(c + 1) * 128],
                                     xall[r0:r0 + EA, s, :], start=True, stop=False)

            def dr(j, half, lo, hi):
                pt = pairs[(j, half)]
                for gi in range(2):
                    gate = half * 2 + gi
                    c = gate * 4 + j
                    nc.tensor.matmul(pt[:, gi, :],
                                     Whh8[:, lo:hi, c * 128:(c + 1) * 128],
                                     Hp8[:, lo:hi, :], start=False,
                                     stop=(hi == HK), perf_mode=DR)

            def acts(j):
                # T layout [128, gate, chunk%2, B]: each gate's two chunks
                # are contiguous, so the whole chunk-pair tail chain runs as
                # four [128, 2, B] DVE ops instead of eight chunk ops
                T = Ts[j // 2 * 2]
                if T is None:
                    T = gsb.tile([128, 4, 2, B], bf16d, tag="T", name="T")
                    Ts[j // 2 * 2] = T
                c = j % 2
                nc.scalar.activation(T[:, 0:2, c, :], pairs[(j, 0)][:],
                                     AF.Tanh, scale=0.5 / SF)
                nc.scalar.activation(T[:, 2:4, c, :], pairs[(j, 1)][:],
                                     AF.Tanh, scale=0.5 / SF)

            def chainP(jlo):  # m1/m2/S' per chunk (short serial latency)
                T = Ts[jlo]
                for c in (0, 1):
                    j = jlo + c
                    m1 = msb.tile([128, B], bf16d, tag="m1", name="m1")
                    nc.vector.scalar_tensor_tensor(m1[:], T[:, 1, c, :], 1.0,
                                                   Sp[:, j, :],
                                                   ALU.add, ALU.mult)
                    m2 = msb.tile([128, B], bf16d, tag="m2", name="m2")
                    nc.vector.scalar_tensor_tensor(m2[:], T[:, 0, c, :], 1.0,
                                                   T[:, 2, c, :],
                                                   ALU.add, ALU.mult)
                    nc.vector.scalar_tensor_tensor(Sn[:, j, :], m1[:],
                                                   0.5, m2[:],
                                                   ALU.mult, ALU.add)

            def thH(jlo):  # tanh(c) for the chunk pair, then per-chunk H'
                th = msb.tile([128, 2, B], bf16d, tag="th", name="th")
                nc.scalar.activation(th[:], Sn[:, jlo:jlo + 2, :],
                                     AF.Tanh, scale=0.5)
                for c in (0, 1):
                    nc.vector.scalar_tensor_tensor(Hn8[:, jlo + c, :],
                                                   Ts[jlo][:, 3, c, :], 1.0,
                                                   th[:, c, :], ALU.add, ALU.mult)

            Ts = {0: None, 2: None}
            # 2-bank pair tiles, 4 in flight: two chunks of PE runway so the
            # ACT read latency never gaps the PE (keeps HAM warm). x-matmuls
            # of chunks 0/1 are H-independent and absorb the previous step's
            # tail; the tail chain runs at chunk-pair granularity and lags
            # so it never head-of-line blocks the gate ACTs.
            xmms(0, 0); xmms(0, 1); xmms(1, 0); xmms(1, 1)
            dr(0, 0, 0, 2); dr(0, 0, 2, 4); dr(0, 1, 0, 2); dr(0, 1, 2, 4)
            acts(0)
            dr(1, 0, 0, 2); dr(1, 0, 2, 4); dr(1, 1, 0, 2); dr(1, 1, 2, 4)
            acts(1)
            xmms(2, 0); xmms(2, 1)
            dr(2, 0, 0, 2); dr(2, 0, 2, 4); dr(2, 1, 0, 2); dr(2, 1, 2, 4)
            acts(2)
            chainP(0)
            thH(0)
            xmms(3, 0); xmms(3, 1)
            dr(3, 0, 0, 2); dr(3, 0, 2, 4); dr(3, 1, 0, 2); dr(3, 1, 2, 4)
            acts(3)
            chainP(2)
            # tail-critical pair: per-chunk th so H'_2 / H'_3 land earlier
            for j in (2, 3):
                th1 = msb.tile([128, B], bf16d, tag="th1", name="th1")
                nc.scalar.activation(th1[:], Sn[:, j, :], AF.Tanh, scale=0.5)
                nc.vector.scalar_tensor_tensor(Hn8[:, j, :],
                                               Ts[2][:, 3, j - 2, :], 1.0,
                                               th1[:], ALU.add, ALU.mult)
            return Hn8, Sn

        # ======== Phase 2: encoder ========
        Hp8 = spool.tile([128, HK, B], f8, tag="H", name="H")
        Sp = spool.tile([128, HK, B], bf16d, tag="S", name="S")
        nc.vector.memset(Hp8[:], 0.2)
        nc.vector.memset(Sp[:], 0.2)
        with tc.tile_pool(name="gpse", bufs=4, space="PSUM") as gps_e:
            for s in range(L):
                Hp8, Sp = lstm_step(gps_e, xenc, s, Hp8, Sp)
        Henc = Hp8

        # ======== Phase 3: t path + attention ========
        with ExitStack() as ph:
            asb = ph.enter_context(tc.tile_pool(name="asb", bufs=2))
            vstr = ph.enter_context(tc.tile_pool(name="vstr", bufs=3))

            # --- 3a: t = relu(Wenc' @ Henc + benc), column-normalized ---
            # own PSUM scope so the attention loop below gets the banks back
            ph3a = ph.enter_context(ExitStack())
            aps = ph3a.enter_context(tc.tile_pool(name="aps", bufs=1, space="PSUM"))
            sps_pool = ph3a.enter_context(tc.tile_pool(name="sps", bufs=2, space="PSUM"))
            tra = asb.tile([128, HK, B], f32, tag="tra", name="tra", bufs=1)
            sqa = asb.tile([128, HK, B], f32r, tag="tsqa", name="tsqa", bufs=1)
            s_ps = aps.tile([1, B], f32, tag="tsum", name="tsum")
            bc_ps = aps.tile([128, B], f32, tag="tbc", name="tbc")
            for mi in range(HK):
                t_ps = sps_pool.tile([128, B], f32, tag="sims", name="sims")
                nc.tensor.matmul(t_ps[:], Wenc8[:, 0:2, mi * 128:(mi + 1) * 128],
                                 Henc[:, 0:2, :], start=True, stop=False,
                                 perf_mode=DR)
                nc.tensor.matmul(t_ps[:], Wenc8[:, 2:4, mi * 128:(mi + 1) * 128],
                                 Henc[:, 2:4, :], start=False, stop=True,
                                 perf_mode=DR)
                nc.scalar.activation(tra[:, mi, :], t_ps[:], AF.Relu,
                                     scale=1.0 / SF, bias=benc[:, mi:mi + 1])
            pass  # warm_pe removed
            nc.scalar.activation(sqa[:], tra[:], AF.Square)
            for mi in range(HK):
                nc.tensor.matmul(s_ps[:], ones_col[:], sqa[:, mi, :],
                                 start=(mi == 0), stop=(mi == HK - 1))
            lnr = asb.tile([1, B], f32, tag="tlnr", name="tlnr")
            nc.scalar.activation(lnr[:], s_ps[:], AF.Ln)
            rs_r = asb.tile([1, B], f32r, tag="trs", name="trs")
            nc.scalar.activation(rs_r[:], lnr[:], AF.Exp, scale=-0.5,
                                 bias=ln16_t[:])
            nc.tensor.matmul(bc_ps[:], ones_row[:], rs_r[:], start=True, stop=True)
            tnp = [asb.tile([128, 2, B], f8, tag=f"tnp{g}", name=f"tnp{g}", bufs=1)
                   for g in range(2)]
            for mi in range(HK):
                nc.vector.tensor_tensor(tnp[mi // 2][:, mi % 2, :],
                                        tra[:, mi, :], bc_ps[:], ALU.mult)
            ph3a.close()

            # --- 3b: attention. Stream gathered vn pairs, E = 16*exp(sims)
            # in fp8, accumulate h with DoubleRow pairs over batch chunks.
            # 4 sim buffers keep 4 blocks in flight so the PE stays dense.
            hps_pool = ph.enter_context(tc.tile_pool(name="hps", bufs=1, space="PSUM"))
            sps_pool = ph.enter_context(tc.tile_pool(name="sps2", bufs=4, space="PSUM"))
            hua = hps_pool.tile([128, HK, B], f32, tag="hua", name="hua")
            hu_ps = [hua[:, h, :] for h in range(HK)]
            pass  # warm_pe removed
            for ip in range(VCHUNKS // 2):
                vnp_i = vstr.tile([128, 2, H], f8, tag="vni", name="vni", bufs=8)
                Ep = vstr.tile([128, 2, B], f8, tag="E", name="E", bufs=6)
                for t_ in range(2):
                    i = 2 * ip + t_
                    r, b = divmod(i, BK)
                    nc.sync.dma_start(vnp_i[:, t_, :], ag_out[r, b])
                    sim_ps = sps_pool.tile([128, B], f32, tag="sims", name="sims")
                    for g in range(2):
                        nc.tensor.matmul(sim_ps[:],
                                         vnTp[g][:, :, i * 128:(i + 1) * 128],
                                         tnp[g][:], start=(g == 0),
                                         stop=(g == 1), perf_mode=DR)
                    nc.scalar.activation(Ep[:, t_, :], sim_ps[:], AF.Exp,
                                         scale=1.0 / 256, bias=ln16c[:])
                for h in range(HK):
                    nc.tensor.matmul(hu_ps[h],
                                     vnp_i[:, :, h * 128:(h + 1) * 128], Ep[:],
                                     start=(ip == 0),
                                     stop=(ip == VCHUNKS // 2 - 1),
                                     perf_mode=DR)
            # normalize h (x2 for the doubled-state convention) -> decoder init
            s2_full = sps_pool.tile([128, B], f32, tag="sims", name="sims")
            s2_ps = s2_full[0:1, :]
            pass  # warm_pe removed
            squ = asb.tile([128, HK, B], f32r, tag="husq", name="husq")
            nc.scalar.activation(squ[:], hua[:], AF.Square)
            for h in range(HK):
                nc.tensor.matmul(s2_ps, ones_col[:], squ[:, h, :],
                                 start=(h == 0), stop=(h == HK - 1))
            lnr2 = asb.tile([1, B], f32, tag="hulnr", name="hulnr")
            nc.scalar.activation(lnr2[:], s2_ps, AF.Ln)
            rs2 = asb.tile([1, B], f32r, tag="hurs", name="hurs")
            nc.scalar.activation(rs2[:], lnr2[:], AF.Exp, scale=-0.5, bias=ln2_t[:])
            bc2_full = sps_pool.tile([128, B], f32, tag="sims", name="sims")
            nc.tensor.matmul(bc2_full[:], ones_row[:], rs2[:], start=True, stop=True)
            bc2_sb = asb.tile([128, B], f32, tag="bc2sb", name="bc2sb", bufs=1)
            nc.vector.tensor_copy(bc2_sb[:], bc2_full[:])
            # decoder init: c0 = h0, so H0 doubles as the initial cell state
            # (step 0's m1 reads it as fp8 -- validated headroom)
            H0 = spool.tile([128, HK, B], f8, tag="H", name="H")
            for h in range(HK):
                nc.vector.tensor_tensor(H0[:, h, :], hu_ps[h], bc2_sb[:], ALU.mult)

        # ======== Phase 4: decoder ========
        dsb = top.enter_context(tc.tile_pool(name="dsb", bufs=2))
        with tc.tile_pool(name="gpsd", bufs=4, space="PSUM") as gps_d:
            pass  # warm_pe removed
            Hp8, Sp = H0, H0
            for s in range(L):
                Hp8, Sp = lstm_step(gps_d, xdec, s, Hp8, Sp)
                # logitsT [V, B] in bank 0 of a rotating pair tile; Z and
                # target exp(logit) reductions land in bank 1 (partitions 0/32)
                lq = gps_d.tile([128, 2, B], f32, tag="pair", name="pair")
                nc.tensor.matmul(lq[:, 0, :], Wout8[:, 0:2, :], Hp8[:, 0:2, :],
                                 start=True, stop=False, perf_mode=DR)
                nc.tensor.matmul(lq[:, 0, :], Wout8[:, 2:4, :], Hp8[:, 2:4, :],
                                 start=False, stop=True, perf_mode=DR)
                El = dsb.tile([128, B], f32r, tag="El", name="El")
                nc.scalar.activation(El[:], lq[:, 0, :], AF.Exp, scale=1.0 / SF)
                nc.tensor.matmul(lq[0:1, 1, :], ones_col[:], El[:],
                                 start=True, stop=True)
                ztmp = dsb.tile([1, B], f32r, tag="ztmp", name="ztmp")
                nc.vector.tensor_copy(ztmp[:], lq[0:1, 1, :])
                nc.sync.dma_start(Zboth[s:s + 1, 0, :], ztmp[:])
                # exp(target logit) via host one-hot * El (on the otherwise
                # idle gpsimd); ln() undoes the exp in the post-pass
                mk = dsb.tile([128, B], f32r, tag="mk", name="mk")
                nc.gpsimd.tensor_tensor(mk[:], ohall[:, s, :], El[:], ALU.mult)
                nc.tensor.matmul(lq[0:1, 1, :], ones_col[:], mk[:],
                                 start=True, stop=True)
                lttmp = dsb.tile([1, B], f32r, tag="lttmp", name="lttmp")
                nc.vector.tensor_copy(lttmp[:], lq[0:1, 1, :])
                nc.sync.dma_start(Zboth[s:s + 1, 1, :], lttmp[:])

            # ======== Phase 5: final loss ========
            lnB = dsb.tile([L, 2, B], f32r, tag="lnB", name="lnB")
            nc.scalar.activation(lnB[:], Zboth[:], AF.Ln)
            diff = dsb.tile([L, B], f32r, tag="diff", name="diff")
            nc.vector.tensor_tensor(diff[:], lnB[:, 0, :], lnB[:, 1, :],
                                    ALU.subtract)
            fq = gps_d.tile([128, 2, B], f32, tag="pair", name="pair")
            nc.tensor.matmul(fq[0:1, 0, :], ones16[:], diff[:], start=True, stop=True)
            loss_sb = dsb.tile([1, B], f32, tag="losssb", name="losssb")
            nc.vector.tensor_scalar(loss_sb[:], fq[0:1, 0, :], 1.0 / L, None,
                                    ALU.mult)
            nc.sync.dma_start(out_d, loss_sb[:])

    nc.compile()
    return nc


def _prep_inputs(visual_input, text_input, emb, W_ih, W_hh, b_ih, b_hh,
                 W_enc, b_enc, W_out, W_vis):
    import ml_dtypes
    bf = ml_dtypes.bfloat16
    f8n = ml_dtypes.float8_e4m3
    f = np.float32
    vis = np.asarray(visual_input, f)[:, 0, :]              # [4096, 2048]
    text = np.asarray(text_input)
    emb = np.asarray(emb, f)
    visT = np.ascontiguousarray(vis.T)                      # [2048, 4096]
    WvisT = (SF * np.asarray(W_vis, f)).T                   # [2048, 512]
    Wvis8 = np.ascontiguousarray(
        WvisT.reshape(VISK // 2, 2, 128, H).transpose(0, 2, 1, 3)).astype(f8n)

    # x-part weights x64 with bias folded as row 50; g-gate block x2 so all
    # gates share the ACT scale 0.5/64
    b = np.asarray(b_ih, f) + np.asarray(b_hh, f)           # [2048]
    WihT = np.concatenate([np.asarray(W_ih, f).T, b[None, :]], axis=0) * SF
    WihT[:, 2 * H:3 * H] *= 2.0                             # [51, 2048]

    def pack8(WT):                                          # [512, M] -> [128, 4, M]
        M = WT.shape[1]
        return np.ascontiguousarray(
            WT.reshape(HK, 128, M).transpose(1, 0, 2)).astype(f8n)

    WhhT = (0.5 * SF) * np.asarray(W_hh, f).T               # [512, 2048]
    WhhT[:, 2 * H:3 * H] *= 2.0
    Whh8 = pack8(WhhT)
    Wenc8 = pack8((0.5 * SF) * np.asarray(W_enc, f).T)      # [512, 512]
    Wout8 = pack8((0.5 * SF) * np.asarray(W_out, f).T)      # [512, 128]
    benc = np.ascontiguousarray(np.asarray(b_enc, f).reshape(HK, 128).T)

    encx = emb[text.T]                                      # [16, 4096, 50]
    dec_ch = np.concatenate([np.zeros((text.shape[0], 1), text.dtype),
                             text[:, :-1]], axis=1)
    decx = emb[dec_ch.T]                                    # [16, 4096, 50]
    one_row = np.ones((L, 1, B_FULL), f)
    encxT = np.concatenate([encx.transpose(0, 2, 1), one_row], axis=1)
    decxT = np.concatenate([decx.transpose(0, 2, 1), one_row], axis=1)
    encxT = np.ascontiguousarray(encxT)                     # [16, 51, 4096]
    decxT = np.ascontiguousarray(decxT)
    tgt = np.ascontiguousarray(text.T.astype(f))            # [16, 4096]
    iota = np.arange(128, dtype=f).reshape(128, 1)
    # one-hot target masks [L, V, B_FULL]
    oneh = (tgt[:, None, :] == iota.reshape(1, 128, 1)).astype(f)

    in_maps = []
    for c in range(NCORES):
        sl = slice(c * B, (c + 1) * B)
        in_maps.append({
            "visT": np.ascontiguousarray(
                visT[:, sl].reshape(VISK // 2, 2, 128, B).transpose(0, 2, 1, 3)
            ).astype(f8n),
            "WvisT": Wvis8,
            "WihT": WihT.astype(bf),
            "Whh8": Whh8,
            "encx": np.ascontiguousarray(encxT[:, :, sl]).astype(bf),
            "decx": np.ascontiguousarray(decxT[:, :, sl]).astype(bf),
            "Wenc8": Wenc8,
            "benc": benc,
            "Wout8": Wout8,
            "tgt": np.ascontiguousarray(tgt[:, sl]),
            "iota128": iota,
            "oneh": np.ascontiguousarray(oneh[:, :, sl]).astype(f8n),
            "ones128": np.ones((128, 1), np.float32),
        })
    return in_maps


LAST_EXEC_TIME_NS = None


def kernel(**inputs):
    global LAST_EXEC_TIME_NS
    from concourse.bass_utils import run_bass_kernel_spmd

    if "nc" not in _CACHE:
        _CACHE["nc"] = _build()
    nc = _CACHE["nc"]
    in_maps = _prep_inputs(**inputs)

    trace = bool(int(os.environ.get("KERNEL_PROFILE", "0")))
    kw = {}
    if trace:
        _install_profile_hook()
        kw["trace"] = True
    res = run_bass_kernel_spmd(nc, in_maps, core_ids=list(range(NCORES)), **kw)
    LAST_EXEC_TIME_NS = res.exec_time_ns
    out = np.concatenate([res.results[c]["loss"][0] for c in range(NCORES)])
    return out.astype(np.float32)


def _install_profile_hook():
    """Optional NTFF profiling (dev only; used when KERNEL_PROFILE=1)."""
    import types, ctypes, contextlib
    try:
        import antenv
    except ImportError:
        return
    if getattr(antenv, "axon_hooks", None) is not None:
        return
    mod = types.ModuleType('antenv.axon_hooks')
    _store = [None]
    mod.set_axon_ntff_profile_hook = lambda h: _store.__setitem__(0, h)
    mod.get_axon_ntff_profile_hook = lambda: _store[0]
    sys.modules['antenv.axon_hooks'] = mod
    antenv.axon_hooks = mod
    try:
        lib = ctypes.CDLL('/opt/axon/libaxon_pjrt.so')
    except OSError:
        return
    if not hasattr(lib, 'axon_start_nrt_profile'):
        return
    lib.axon_start_nrt_profile.argtypes = [ctypes.POINTER(ctypes.c_int64),
                                           ctypes.c_size_t]
    lib.axon_start_nrt_profile.restype = ctypes.c_int64
    lib.axon_stop_nrt_profile.argtypes = [ctypes.c_char_p]
    lib.axon_stop_nrt_profile.restype = ctypes.c_int64

    @contextlib.contextmanager
    def _hook(output_dir, device_ids):
        import jax
        jax.devices()
        if device_ids:
            ids = (ctypes.c_int64 * len(device_ids))(*device_ids)
            rc = lib.axon_start_nrt_profile(ids, len(device_ids))
        else:
            rc = lib.axon_start_nrt_profile(None, 0)
        if rc != 0:
            raise RuntimeError(f"axon_start_nrt_profile rc={rc}")
        try:
            yield
        finally:
            n = lib.axon_stop_nrt_profile(str(output_dir).encode())
            print(f"profile: {n} ntff file(s) in {output_dir}", file=sys.stderr)

    mod.set_axon_ntff_profile_hook = mod.set_axon_ntff_profile_hook
    mod.set_axon_ntff_profile_hook(_hook)
    import concourse.bass_utils as bu
    bu.upload_artifacts = lambda tmpdir: "local://" + str(tmpdir)


# revision 103
# speedup vs baseline: 1.1446x; 1.1446x over previous
"""Trainium2 Bass kernel for nn_AttentionModelCharLevel.

Model (per reference): visual linear -> char-encoder LSTM -> linear+relu ->
cosine attention (softmax over batch dim) -> char-decoder LSTM -> per-sample
mean NLL over L steps.

Sharding: data-parallel over batch B=4096 across 8 cores (512 rows each).
The [B,B] attention needs every core to see all normalized visual rows, so
each core computes its vn shard (+transpose), AllGathers both, and streams
the gathered rows back through SBUF during the attention phase.

Key device-side conventions (v2, fp8 DoubleRow):
- The LSTM recurrence, encoder linear and decoder logits matmuls run in
  fp8e4 with perf_mode=DoubleRow (2 fp8 K-rows per PE cell): K=512
  contractions become 2 matmuls of logical K=256. Weights are pre-scaled
  x64 host-side (fp8 normal range) and the x1/64 is folded into the ACT
  scale that reads the PSUM.
- Hidden state is stored doubled (Ht = 2h) as a single [128, 4, B] fp8
  tile per step; slice [:, 2g:2g+2, :] is the DoubleRow rhs pair. Cell
  state St = 2c is a [128, 4, B] bf16 tile.
- sigmoid(z) = 0.5*tanh(z/2)+0.5; the g-gate's weights carry an extra x2
  so ALL gates share one ACT scale (0.5/64). The bias is folded into the
  x-part matmul as a 51st embedding row (x=1), so one ACT with no bias
  covers a whole 4-bank PSUM quad [128, 4, B] = the 4 gates of one hidden
  chunk -> 4 gate ACTs + 1 tanh(c) ACT per step instead of 20.
- The DVE tail chain per hidden chunk runs in bf16 (2x DVE rate):
      m2 = (tanh_i + 1) * tanh_g
      m1 = (tanh_f + 1) * S
      S' = 0.5*m1 + m2
      H' = (tanh_o + 1) * tanh(0.5*S')   (fp8 out)
- Softmax over the batch dim reduces to exp() only: sims are cosine
  similarities in [-1,1] and the softmax denominator is a positive
  per-column scale that the final row normalization of h cancels.
- Decoder log-softmax: logits kept in [V, B] layout; Z = ones^T exp(logits)
  and the target logit via a one-hot mask built from an iota column --
  both reduce over partitions with K=128 ones-matmuls into spare banks of
  the rotating PSUM quads. ln() deferred to a single post-pass.
"""
import os
import sys

sys.path.insert(0, '/opt/trn_rl_repo')

import numpy as np

B_FULL = 4096
NCORES = 8
B = B_FULL // NCORES          # 512 rows per core
H = 512
G = 4 * H                     # 2048
E = 50
EA = E + 1                    # embedding dim + bias row
L = 16
V = 128
VIS = 2048
HK = H // 128                 # 4 chunks of the hidden dim
GK = G // 128                 # 16 gate chunks
VISK = VIS // 128             # 16 chunks of the visual dim
BK = B // 128                 # 4 batch chunks per core
VCHUNKS = B_FULL // 128       # 32 chunks of the full batch
SF = 64.0                     # fp8 weight scale

_CACHE = {}


def _build():
    import concourse.bass as bass
    import concourse.tile as tile
    import concourse.mybir as mybir
    from concourse import bacc
    from concourse.masks import make_identity
    from contextlib import ExitStack

    dt = mybir.dt
    AF = mybir.ActivationFunctionType
    ALU = mybir.AluOpType
    DR = mybir.MatmulPerfMode.DoubleRow
    f32 = dt.float32
    f32r = dt.float32r
    bf16d = dt.bfloat16
    f8 = dt.float8e4
    LN2 = float(np.log(2.0))

    AP = bass.AP
    nc = bacc.Bacc("TRN2", target_bir_lowering=False, debug=False,
                   num_devices=NCORES)

    # ---- DRAM I/O ----
    visT_d = nc.dram_tensor("visT", [VISK // 2, 128, 2, B], f8, kind="ExternalInput").ap()
    WvisT_d = nc.dram_tensor("WvisT", [VISK // 2, 128, 2, H], f8, kind="ExternalInput").ap()
    WihT_d = nc.dram_tensor("WihT", [EA, G], bf16d, kind="ExternalInput").ap()
    Whh8_d = nc.dram_tensor("Whh8", [128, HK, G], f8, kind="ExternalInput").ap()
    encx_d = nc.dram_tensor("encx", [L, EA, B], bf16d, kind="ExternalInput").ap()
    decx_d = nc.dram_tensor("decx", [L, EA, B], bf16d, kind="ExternalInput").ap()
    Wenc8_d = nc.dram_tensor("Wenc8", [128, HK, H], f8, kind="ExternalInput").ap()
    benc_d = nc.dram_tensor("benc", [128, HK], f32, kind="ExternalInput").ap()
    Wout8_d = nc.dram_tensor("Wout8", [128, HK, V], f8, kind="ExternalInput").ap()
    tgt_d = nc.dram_tensor("tgt", [L, B], f32, kind="ExternalInput").ap()
    iota_d = nc.dram_tensor("iota128", [128, 1], f32, kind="ExternalInput").ap()
    oneh_d = nc.dram_tensor("oneh", [L, 128, B], f8, kind="ExternalInput").ap()
    ones_d = nc.dram_tensor("ones128", [128, 1], f32r, kind="ExternalInput").ap()
    out_d = nc.dram_tensor("loss", [1, B], f32, kind="ExternalOutput").ap()

    with tile.TileContext(nc) as tc, ExitStack() as top:
        wpool = top.enter_context(tc.tile_pool(name="w", bufs=1))
        spool = top.enter_context(tc.tile_pool(name="state", bufs=2))
        dram = top.enter_context(tc.tile_pool(name="dram", bufs=1, space="DRAM"))

        # ---- persistent weights / constants (DMAs issued after the visual
        # input DMAs: the scan weights aren't needed until the encoder) ----
        Whh8 = wpool.tile([128, HK, G], f8, tag="whh8", name="whh8")
        WihT2 = wpool.tile([64 + EA, G], bf16d, tag="wih", name="wih")
        Wenc8 = wpool.tile([128, HK, H], f8, tag="wenc8", name="wenc8")
        benc = wpool.tile([128, HK], f32, tag="benc", name="benc")
        Wout8 = wpool.tile([128, HK, V], f8, tag="wout8", name="wout8")

        def load_weights():
            nc.sync.dma_start(Whh8[:], Whh8_d)
            nc.sync.dma_start(WihT2[:EA, :], WihT_d)
            nc.sync.dma_start(WihT2[64:64 + EA, :], WihT_d)
            nc.sync.dma_start(Wenc8[:], Wenc8_d)
            nc.sync.dma_start(benc[:], benc_d)
            nc.sync.dma_start(Wout8[:], Wout8_d)

        ones_col = wpool.tile([128, 1], f32r, tag="ones_col", name="ones_col")
        nc.sync.dma_start(ones_col[:], ones_d)
        ones_row = wpool.tile([1, 128], f32r, tag="ones_row", name="ones_row")
        nc.sync.dma_start(ones_row[:], ones_d.rearrange("p one -> one p"))
        ones16 = wpool.tile([L, 1], f32r, tag="ones16", name="ones16")
        nc.sync.dma_start(ones16[:], ones_d[:L])
        ident8 = wpool.tile([128, 128], f8, tag="ident8", name="ident8")
        make_identity(nc, ident8[:])

        def warm_pe(out_ap, n):
            # dummy matmuls into a region the next real start=True matmul
            # resets; they run in otherwise-idle PE windows and keep the
            # HAM activity window busy so real matmuls stay at 2.4 GHz
            for _ in range(n):
                nc.tensor.matmul(out_ap, ident8[:], ident8[:],
                                 start=True, stop=True,
                                 skip_group_check=True)
        ln2_t = wpool.tile([1, 1], f32, tag="ln2", name="ln2")
        nc.vector.memset(ln2_t[:], LN2)
        ln16_t = wpool.tile([1, 1], f32, tag="ln16", name="ln16")
        nc.vector.memset(ln16_t[:], float(np.log(16.0)))
        ln16c = wpool.tile([128, 1], f32, tag="ln16c", name="ln16c")
        nc.vector.memset(ln16c[:], float(np.log(16.0)))

        # prefetch ALL scan inputs up front on the gpsimd DMA queue so the
        # LSTM scans never wait on the sync queue / collective window
        xenc = wpool.tile([64 + EA, L, B], bf16d, tag="xenc", name="xenc")
        xdec = wpool.tile([64 + EA, L, B], bf16d, tag="xdec", name="xdec")
        for s in range(L):
            nc.gpsimd.dma_start(xenc[:EA, s, :], encx_d[s])
            nc.gpsimd.dma_start(xenc[64:64 + EA, s, :], encx_d[s])
        for s in range(L):
            nc.gpsimd.dma_start(xdec[:EA, s, :], decx_d[s])
            nc.gpsimd.dma_start(xdec[64:64 + EA, s, :], decx_d[s])
        ohall = wpool.tile([128, L, B], f8, tag="ohall", name="ohall")
        nc.gpsimd.dma_start(ohall[:], oneh_d.rearrange("l p b -> p l b"))

        # AllGather buffers: vn blocks and vnT blocks (both fp8, x16 scale)
        ag_in = dram.tile([BK, 128, B], f8, name="ag_in")
        ag_out = dram.tile([NCORES, BK, 128, B], f8, addr_space="Shared", name="ag_out")
        agt_in = dram.tile([HK, 128, B], f8, name="agt_in")
        agt_out = dram.tile([NCORES, HK, 128, B], f8, addr_space="Shared", name="agt_out")

        # decoder per-step Z and target-logit rows
        zpool = top.enter_context(tc.tile_pool(name="zp", bufs=1))
        Zboth = zpool.tile([L, 2, B], f32r, tag="Zb", name="Zb")
        Zs = Zboth[:, 0, :]
        lts = Zboth[:, 1, :]

        # ======== Phase 1: visual linear + row-normalize + transpose + AG ====
        with ExitStack() as ph:
            vsb = ph.enter_context(tc.tile_pool(name="vsb", bufs=3))
            vps = ph.enter_context(tc.tile_pool(name="vps", bufs=1, space="PSUM"))
            tps = ph.enter_context(tc.tile_pool(name="tps", bufs=4, space="PSUM"))
            vnpool = ph.enter_context(tc.tile_pool(name="vnp", bufs=1))

            v_ps = [vps.tile([128, H], f32, tag=f"vps{b}", name=f"vps{b}") for b in range(BK)]
            pass  # warm_pe removed
            for ki in range(VISK // 2):
                vis_t = vsb.tile([128, 2, B], f8, tag="vis", name="vis")
                nc.sync.dma_start(vis_t[:], visT_d[ki])
                wv_t = vsb.tile([128, 2, H], f8, tag="wvis", name="wvis")
                nc.sync.dma_start(wv_t[:], WvisT_d[ki])
                for b in range(BK):
                    nc.tensor.matmul(v_ps[b][:], vis_t[:, :, b * 128:(b + 1) * 128],
                                     wv_t[:], start=(ki == 0),
                                     stop=(ki == VISK // 2 - 1), perf_mode=DR)
            load_weights()
            # batched by function so the ACT table isn't thrashed
            s_cols, vn = [], []
            for b in range(BK):
                sq = vsb.tile([128, H], f32, tag="vsq", name="vsq")
                s_col = vsb.tile([128, 1], f32, tag=f"vscol{b}", name=f"vscol{b}", bufs=1)
                nc.scalar.activation(sq[:], v_ps[b][:], AF.Square,
                                     accum_out=s_col[:])
                s_cols.append(s_col)
            lns = []
            for b in range(BK):
                lnc_ = vsb.tile([128, 1], f32, tag=f"vln{b}", name=f"vln{b}", bufs=1)
                nc.scalar.activation(lnc_[:], s_cols[b][:], AF.Ln)
                lns.append(lnc_)
            # sqrt(s) = 64*|v|, so exp(-0.5*ln s + ln16) puts vn in fp8 x16
            rss = []
            for b in range(BK):
                rs = vsb.tile([128, 1], f32, tag=f"vrs{b}", name=f"vrs{b}", bufs=1)
                nc.scalar.activation(rs[:], lns[b][:], AF.Exp, scale=-0.5,
                                     bias=ln16c[:])
                rss.append(rs)
            for b in range(BK):
                vn_b = vnpool.tile([128, H], f8, tag=f"vn{b}", name=f"vn{b}")
                nc.vector.tensor_scalar(vn_b[:], v_ps[b][:], rss[b][:], None, ALU.mult)
                vn.append(vn_b)
            # transpose vn -> vnT (16 128x128 blocks, fp8 for the sims lhsT)
            vnT = [vnpool.tile([128, B], f8, tag=f"vnT{h}", name=f"vnT{h}") for h in range(HK)]
            for b in range(BK):
                for h in range(HK):
                    t_ps = tps.tile([128, 128, 2], f8, tag="tr", name="tr")
                    nc.tensor.transpose(
                        t_ps[:, :, 0], vn[b][:, h * 128:(h + 1) * 128], ident8[:])
                    nc.vector.tensor_copy(vnT[h][:, b * 128:(b + 1) * 128],
                                          t_ps[:, :, 0])
            for b in range(BK):
                nc.sync.dma_start(ag_in[b], vn[b][:])
            for h in range(HK):
                nc.sync.dma_start(agt_in[h], vnT[h][:])
            nc.gpsimd.collective_compute(
                "AllGather", mybir.AluOpType.bypass,
                replica_groups=[list(range(NCORES))],
                ins=[ag_in[:]], outs=[ag_out[:]],
            )
            nc.gpsimd.collective_compute(
                "AllGather", mybir.AluOpType.bypass,
                replica_groups=[list(range(NCORES))],
                ins=[agt_in[:]], outs=[agt_out[:]],
            )

        # staged full vnT (fp8, DoubleRow pair layout) for the sims lhsT;
        # filled by gpsimd DMAs that wait on the collective during the encoder
        vnTp = [wpool.tile([128, 2, B_FULL], f8, tag=f"vnTp{g}", name=f"vnTp{g}")
                for g in range(2)]
        for k in range(HK):
            for r in range(NCORES):
                nc.gpsimd.dma_start(vnTp[k // 2][:, k % 2, r * B:(r + 1) * B],
                                    agt_out[r, k])

        # ======== LSTM scan helper ========
        gsb = top.enter_context(tc.tile_pool(name="gsb", bufs=3))
        msb = top.enter_context(tc.tile_pool(name="msb", bufs=4))

        from collections import deque

        def lstm_step(gps, xall, s, Hp8, Sp, max_open=2):
            """One LSTM step, fp8 DoubleRow recurrence, quad-bank PSUM.

            Per hidden chunk j one 4-bank PSUM quad holds gates i,f,g,o.
            open = x-part matmuls (K=51, two concurrent row groups) plus the
            h01 DoubleRow pair; close = h23 pair + one mega-ACT over the quad
            + the state chain split across gpsimd (m2, m1) and DVE (S', H'),
            with tanh(c) in chunk pairs on ACT. Closes lag opens so the PE
            streams the next chunk while the previous drains, and the first
            closed chunks feed the next step's opens.
            """
            Hn8 = spool.tile([128, HK, B], f8, tag="H", name="H")
            Sn = spool.tile([128, HK, B], bf16d, tag="S", name="S")
            Ts = {}
            pairs = {}

            def xmms(j, half):
                pt = gps.tile([128, 2, B], f32, tag="pair", name="pair")
                pairs[(j, half)] = pt
                for gi in range(2):
                    gate = half * 2 + gi
                    c = gate * 4 + j
                    r0 = 0 if gi == 0 else 64
                    nc.tensor.matmul(pt[:, gi, :],
                                     WihT2[r0:r0 + EA, c * 128:(c + 1) * 128],
                                     xall[r0:r0 + EA, s, :], start=True, stop=False)

            def dr(j, half, lo, hi):
                pt = pairs[(j, half)]
                for gi in range(2):
                    gate = half * 2 + gi
                    c = gate * 4 + j
                    nc.tensor.matmul(pt[:, gi, :],
                                     Whh8[:, lo:hi, c * 128:(c + 1) * 128],
                                     Hp8[:, lo:hi, :], start=False,
                                     stop=(hi == HK), perf_mode=DR)

            def acts(j):
                # T layout [128, gate, chunk%2, B]: each gate's two chunks
                # are contiguous, so the whole chunk-pair tail chain runs as
                # four [128, 2, B] DVE ops instead of eight chunk ops
                T = Ts[j // 2 * 2]
                if T is None:
                    T = gsb.tile([128, 4, 2, B], bf16d, tag="T", name="T")
                    Ts[j // 2 * 2] = T
                c = j % 2
                nc.scalar.activation(T[:, 0:2, c, :], pairs[(j, 0)][:],
                                     AF.Tanh, scale=0.5 / SF)
                nc.scalar.activation(T[:, 2:4, c, :], pairs[(j, 1)][:],
                                     AF.Tanh, scale=0.5 / SF)

            def chainP(jlo):  # m1/m2/S' per chunk (short serial latency)
                T = Ts[jlo]
                for c in (0, 1):
                    j = jlo + c
                    m1 = msb.tile([128, B], bf16d, tag="m1", name="m1")
                    nc.vector.scalar_tensor_tensor(m1[:], T[:, 1, c, :], 1.0,
                                                   Sp[:, j, :],
                                                   ALU.add, ALU.mult)
                    m2 = msb.tile([128, B], bf16d, tag="m2", name="m2")
                    nc.vector.scalar_tensor_tensor(m2[:], T[:, 0, c, :], 1.0,
                                                   T[:, 2, c, :],
                                                   ALU.add, ALU.mult)
                    nc.vector.scalar_tensor_tensor(Sn[:, j, :], m1[:],
                                                   0.5, m2[:],
                                                   ALU.mult, ALU.add)

            def thH(jlo):  # tanh(c) for the chunk pair, then per-chunk H'
                th = msb.tile([128, 2, B], bf16d, tag="th", name="th")
                nc.scalar.activation(th[:], Sn[:, jlo:jlo + 2, :],
                                     AF.Tanh, scale=0.5)
                for c in (0, 1):
                    nc.vector.scalar_tensor_tensor(Hn8[:, jlo + c, :],
                                                   Ts[jlo][:, 3, c, :], 1.0,
                                                   th[:, c, :], ALU.add, ALU.mult)

            Ts = {0: None, 2: None}
            # 2-bank pair tiles, 4 in flight: two chunks of PE runway so the
            # ACT read latency never gaps the PE (keeps HAM warm). x-matmuls
            # of chunks 0/1 are H-independent and absorb the previous step's
            # tail; the tail chain runs at chunk-pair granularity and lags
            # so it never head-of-line blocks the gate ACTs.
            xmms(0, 0); xmms(0, 1); xmms(1, 0); xmms(1, 1)
            dr(0, 0, 0, 2); dr(0, 0, 2, 4); dr(0, 1, 0, 2); dr(0, 1, 2, 4)
            acts(0)
            dr(1, 0, 0, 2); dr(1, 0, 2, 4); dr(1, 1, 0, 2); dr(1, 1, 2, 4)
            acts(1)
            xmms(2, 0); xmms(2, 1)
            dr(2, 0, 0, 2); dr(2, 0, 2, 4); dr(2, 1, 0, 2); dr(2, 1, 2, 4)
            acts(2)
            chainP(0)
            thH(0)
            xmms(3, 0); xmms(3, 1)
            dr(3, 0, 0, 2); dr(3, 0, 2, 4); dr(3, 1, 0, 2); dr(3, 1, 2, 4)
            acts(3)
            chainP(2)
            # tail-critical pair: per-chunk th so H'_2 / H'_3 land earlier
            for j in (2, 3):
                th1 = msb.tile([128, B], bf16d, tag="th1", name="th1")
                nc.scalar.activation(th1[:], Sn[:, j, :], AF.Tanh, scale=0.5)
                nc.vector.scalar_tensor_tensor(Hn8[:, j, :],
                                               Ts[2][:, 3, j - 2, :], 1.0,
                                               th1[:], ALU.add, ALU.mult)
            return Hn8, Sn

        # ======== Phase 2: encoder ========
        Hp8 = spool.tile([128, HK, B], f8, tag="H", name="H")
        Sp = spool.tile([128, HK, B], bf16d, tag="S", name="S")
        nc.vector.memset(Hp8[:], 0.2)
        nc.vector.memset(Sp[:], 0.2)
        with tc.tile_pool(name="gpse", bufs=4, space="PSUM") as gps_e:
            for s in range(L):
                Hp8, Sp = lstm_step(gps_e, xenc, s, Hp8, Sp)
        Henc = Hp8

        # ======== Phase 3: t path + attention ========
        with ExitStack() as ph:
            asb = ph.enter_context(tc.tile_pool(name="asb", bufs=2))
            vstr = ph.enter_context(tc.tile_pool(name="vstr", bufs=3))

            # --- 3a: t = relu(Wenc' @ Henc + benc), column-normalized ---
            # own PSUM scope so the attention loop below gets the banks back
            ph3a = ph.enter_context(ExitStack())
            aps = ph3a.enter_context(tc.tile_pool(name="aps", bufs=1, space="PSUM"))
            sps_pool = ph3a.enter_context(tc.tile_pool(name="sps", bufs=2, space="PSUM"))
            tra = asb.tile([128, HK, B], f32, tag="tra", name="tra", bufs=1)
            sqa = asb.tile([128, HK, B], f32r, tag="tsqa", name="tsqa", bufs=1)
            s_ps = aps.tile([1, B], f32, tag="tsum", name="tsum")
            bc_ps = aps.tile([128, B], f32, tag="tbc", name="tbc")
            for mi in range(HK):
                t_ps = sps_pool.tile([128, B], f32, tag="sims", name="sims")
                nc.tensor.matmul(t_ps[:], Wenc8[:, 0:2, mi * 128:(mi + 1) * 128],
                                 Henc[:, 0:2, :], start=True, stop=False,
                                 perf_mode=DR)
                nc.tensor.matmul(t_ps[:], Wenc8[:, 2:4, mi * 128:(mi + 1) * 128],
                                 Henc[:, 2:4, :], start=False, stop=True,
                                 perf_mode=DR)
                nc.scalar.activation(tra[:, mi, :], t_ps[:], AF.Relu,
                                     scale=1.0 / SF, bias=benc[:, mi:mi + 1])
            pass  # warm_pe removed
            nc.scalar.activation(sqa[:], tra[:], AF.Square)
            for mi in range(HK):
                nc.tensor.matmul(s_ps[:], ones_col[:], sqa[:, mi, :],
                                 start=(mi == 0), stop=(mi == HK - 1))
            lnr = asb.tile([1, B], f32, tag="tlnr", name="tlnr")
            nc.scalar.activation(lnr[:], s_ps[:], AF.Ln)
            rs_r = asb.tile([1, B], f32r, tag="trs", name="trs")
            nc.scalar.activation(rs_r[:], lnr[:], AF.Exp, scale=-0.5,
                                 bias=ln16_t[:])
            nc.tensor.matmul(bc_ps[:], ones_row[:], rs_r[:], start=True, stop=True)
            tnp = [asb.tile([128, 2, B], f8, tag=f"tnp{g}", name=f"tnp{g}", bufs=1)
                   for g in range(2)]
            for mi in range(HK):
                nc.vector.tensor_tensor(tnp[mi // 2][:, mi % 2, :],
                                        tra[:, mi, :], bc_ps[:], ALU.mult)
            ph3a.close()

            # --- 3b: attention. Stream gathered vn pairs, E = 16*exp(sims)
            # in fp8, accumulate h with DoubleRow pairs over batch chunks.
            # 4 sim buffers keep 4 blocks in flight so the PE stays dense.
            hps_pool = ph.enter_context(tc.tile_pool(name="hps", bufs=1, space="PSUM"))
            sps_pool = ph.enter_context(tc.tile_pool(name="sps2", bufs=4, space="PSUM"))
            hua = hps_pool.tile([128, HK, B], f32, tag="hua", name="hua")
            hu_ps = [hua[:, h, :] for h in range(HK)]
            pass  # warm_pe removed
            for ip in range(VCHUNKS // 2):
                vnp_i = vstr.tile([128, 2, H], f8, tag="vni", name="vni", bufs=8)
                Ep = vstr.tile([128, 2, B], f8, tag="E", name="E", bufs=6)
                for t_ in range(2):
                    i = 2 * ip + t_
                    r, b = divmod(i, BK)
                    nc.sync.dma_start(vnp_i[:, t_, :], ag_out[r, b])
                    sim_ps = sps_pool.tile([128, B], f32, tag="sims", name="sims")
                    for g in range(2):
                        nc.tensor.matmul(sim_ps[:],
                                         vnTp[g][:, :, i * 128:(i + 1) * 128],
                                         tnp[g][:], start=(g == 0),
                                         stop=(g == 1), perf_mode=DR)
                    nc.scalar.activation(Ep[:, t_, :], sim_ps[:], AF.Exp,
                                         scale=1.0 / 256, bias=ln16c[:])
                for h in range(HK):
                    nc.tensor.matmul(hu_ps[h],
                                     vnp_i[:, :, h * 128:(h + 1) * 128], Ep[:],
                                     start=(ip == 0),
                                     stop=(ip == VCHUNKS // 2 - 1),
                                     perf_mode=DR)
            # normalize h (x2 for the doubled-state convention) -> decoder init
            s2_full = sps_pool.tile([128, B], f32, tag="sims", name="sims")
            s2_ps = s2_full[0:1, :]
            pass  # warm_pe removed
            squ = asb.tile([128, HK, B], f32r, tag="husq", name="husq")
            nc.scalar.activation(squ[:], hua[:], AF.Square)
            for h in range(HK):
                nc.tensor.matmul(s2_ps, ones_col[:], squ[:, h, :],
                                 start=(h == 0), stop=(h == HK - 1))
            lnr2 = asb.tile([1, B], f32, tag="hulnr", name="hulnr")
            nc.scalar.activation(lnr2[:], s2_ps, AF.Ln)
            rs2 = asb.tile([1, B], f32r, tag="hurs", name="hurs")
            nc.scalar.activation(rs2[:], lnr2[:], AF.Exp, scale=-0.5, bias=ln2_t[:])
            bc2_full = sps_pool.tile([128, B], f32, tag="sims", name="sims")
            nc.tensor.matmul(bc2_full[:], ones_row[:], rs2[:], start=True, stop=True)
            bc2_sb = asb.tile([128, B], f32, tag="bc2sb", name="bc2sb", bufs=1)
            nc.vector.tensor_copy(bc2_sb[:], bc2_full[:])
            # decoder init: c0 = h0, so H0 doubles as the initial cell state
            # (step 0's m1 reads it as fp8 -- validated headroom)
            H0 = spool.tile([128, HK, B], f8, tag="H", name="H")
            for h in range(HK):
                nc.vector.tensor_tensor(H0[:, h, :], hu_ps[h], bc2_sb[:], ALU.mult)

        # ======== Phase 4: decoder ========
        dsb = top.enter_context(tc.tile_pool(name="dsb", bufs=2))
        with tc.tile_pool(name="gpsd", bufs=4, space="PSUM") as gps_d:
            pass  # warm_pe removed
            Hp8, Sp = H0, H0
            for s in range(L):
                Hp8, Sp = lstm_step(gps_d, xdec, s, Hp8, Sp)
                # logitsT [V, B] in bank 0 of a rotating pair tile; Z and
                # target exp(logit) reductions land in bank 1 (partitions 0/32)
                lq = gps_d.tile([128, 2, B], f32, tag="pair", name="pair")
                nc.tensor.matmul(lq[:, 0, :], Wout8[:, 0:2, :], Hp8[:, 0:2, :],
                                 start=True, stop=False, perf_mode=DR)
                nc.tensor.matmul(lq[:, 0, :], Wout8[:, 2:4, :], Hp8[:, 2:4, :],
                                 start=False, stop=True, perf_mode=DR)
                El = dsb.tile([128, B], f32r, tag="El", name="El")
                nc.scalar.activation(El[:], lq[:, 0, :], AF.Exp, scale=1.0 / SF)
                nc.tensor.matmul(lq[0:1, 1, :], ones_col[:], El[:],
                                 start=True, stop=True)
                ztmp = dsb.tile([1, B], f32r, tag="ztmp", name="ztmp")
                nc.vector.tensor_copy(ztmp[:], lq[0:1, 1, :])
                nc.sync.dma_start(Zboth[s:s + 1, 0, :], ztmp[:])
                # exp(target logit) via host one-hot * El (on the otherwise
                # idle gpsimd); ln() undoes the exp in the post-pass
                mk = dsb.tile([128, B], f32r, tag="mk", name="mk")
                nc.gpsimd.tensor_tensor(mk[:], ohall[:, s, :], El[:], ALU.mult)
                nc.tensor.matmul(lq[0:1, 1, :], ones_col[:], mk[:],
                                 start=True, stop=True)
                lttmp = dsb.tile([1, B], f32r, tag="lttmp", name="lttmp")
                nc.vector.tensor_copy(lttmp[:], lq[0:1, 1, :])
                nc.sync.dma_start(Zboth[s:s + 1, 1, :], lttmp[:])

            # ======== Phase 5: final loss ========
            lnB = dsb.tile([L, 2, B], f32r, tag="lnB", name="lnB")
            nc.scalar.activation(lnB[:], Zboth[:], AF.Ln)
            diff = dsb.tile([L, B], f32r, tag="diff", name="diff")
            nc.vector.tensor_tensor(diff[:], lnB[:, 0, :], lnB[:, 1, :],
                                    ALU.subtract)
            fq = gps_d.tile([128, 2, B], f32, tag="pair", name="pair")
            nc.tensor.matmul(fq[0:1, 0, :], ones16[:], diff[:], start=True, stop=True)
            loss_sb = dsb.tile([1, B], f32, tag="losssb", name="losssb")
            nc.vector.tensor_scalar(loss_sb[:], fq[0:1, 0, :], 1.0 / L, None,
                                    ALU.mult)
            nc.sync.dma_start(out_d, loss_sb[:])

    nc.compile()
    return nc


def _prep_inputs(visual_input, text_input, emb, W_ih, W_hh, b_ih, b_hh,
                 W_enc, b_enc, W_out, W_vis):
    import ml_dtypes
    bf = ml_dtypes.bfloat16
    f8n = ml_dtypes.float8_e4m3
    f = np.float32
    vis = np.asarray(visual_input, f)[:, 0, :]              # [4096, 2048]
    text = np.asarray(text_input)
    emb = np.asarray(emb, f)
    visT = np.ascontiguousarray(vis.T)                      # [2048, 4096]
    WvisT = (SF * np.asarray(W_vis, f)).T                   # [2048, 512]
    Wvis8 = np.ascontiguousarray(
        WvisT.reshape(VISK // 2, 2, 128, H).transpose(0, 2, 1, 3)).astype(f8n)

    # x-part weights x64 with bias folded as row 50; g-gate block x2 so all
    # gates share the ACT scale 0.5/64
    b = np.asarray(b_ih, f) + np.asarray(b_hh, f)           # [2048]
    WihT = np.concatenate([np.asarray(W_ih, f).T, b[None, :]], axis=0) * SF
    WihT[:, 2 * H:3 * H] *= 2.0                             # [51, 2048]

    def pack8(WT):                                          # [512, M] -> [128, 4, M]
        M = WT.shape[1]
        return np.ascontiguousarray(
            WT.reshape(HK, 128, M).transpose(1, 0, 2)).astype(f8n)

    WhhT = (0.5 * SF) * np.asarray(W_hh, f).T               # [512, 2048]
    WhhT[:, 2 * H:3 * H] *= 2.0
    Whh8 = pack8(WhhT)
    Wenc8 = pack8((0.5 * SF) * np.asarray(W_enc, f).T)      # [512, 512]
    Wout8 = pack8((0.5 * SF) * np.asarray(W_out, f).T)      # [512, 128]
    benc = np.ascontiguousarray(np.asarray(b_enc, f).reshape(HK, 128).T)

    encx = emb[text.T]                                      # [16, 4096, 50]
    dec_ch = np.concatenate([np.zeros((text.shape[0], 1), text.dtype),
                             text[:, :-1]], axis=1)
    decx = emb[dec_ch.T]                                    # [16, 4096, 50]
    one_row = np.ones((L, 1, B_FULL), f)
    encxT = np.concatenate([encx.transpose(0, 2, 1), one_row], axis=1)
    decxT = np.concatenate([decx.transpose(0, 2, 1), one_row], axis=1)
    encxT = np.ascontiguousarray(encxT)                     # [16, 51, 4096]
    decxT = np.ascontiguousarray(decxT)
    tgt = np.ascontiguousarray(text.T.astype(f))            # [16, 4096]
    iota = np.arange(128, dtype=f).reshape(128, 1)
    # one-hot target masks [L, V, B_FULL]
    oneh = (tgt[:, None, :] == iota.reshape(1, 128, 1)).astype(f)

    in_maps = []
    for c in range(NCORES):
        sl = slice(c * B, (c + 1) * B)
        in_maps.append({
            "visT": np.ascontiguousarray(
                visT[:, sl].reshape(VISK // 2, 2, 128, B).transpose(0, 2, 1, 3)
            ).astype(f8n),
            "WvisT": Wvis8,
            "WihT": WihT.astype(bf),
            "Whh8": Whh8,
            "encx": np.ascontiguousarray(encxT[:, :, sl]).astype(bf),
            "decx": np.ascontiguousarray(decxT[:, :, sl]).astype(bf),
            "Wenc8": Wenc8,
            "benc": benc,
            "Wout8": Wout8,
            "tgt": np.ascontiguousarray(tgt[:, sl]),
            "iota128": iota,
            "oneh": np.ascontiguousarray(oneh[:, :, sl]).astype(f8n),
            "ones128": np.ones((128, 1), np.float32),
        })
    return in_maps


LAST_EXEC_TIME_NS = None


def kernel(**inputs):
    global LAST_EXEC_TIME_NS
    from concourse.bass_utils import run_bass_kernel_spmd

    if "nc" not in _CACHE:
        _CACHE["nc"] = _build()
    nc = _CACHE["nc"]
    in_maps = _prep_inputs(**inputs)

    trace = bool(int(os.environ.get("KERNEL_PROFILE", "0")))
    kw = {}
    if trace:
        _install_profile_hook()
        kw["trace"] = True
    res = run_bass_kernel_spmd(nc, in_maps, core_ids=list(range(NCORES)), **kw)
    LAST_EXEC_TIME_NS = res.exec_time_ns
    out = np.concatenate([res.results[c]["loss"][0] for c in range(NCORES)])
    return out.astype(np.float32)


def _install_profile_hook():
    """Optional NTFF profiling (dev only; used when KERNEL_PROFILE=1)."""
    import types, ctypes, contextlib
    try:
        import antenv
    except ImportError:
        return
    if getattr(antenv, "axon_hooks", None) is not None:
        return
    mod = types.ModuleType('antenv.axon_hooks')
    _store = [None]
    mod.set_axon_ntff_profile_hook = lambda h: _store.__setitem__(0, h)
    mod.get_axon_ntff_profile_hook = lambda: _store[0]
    sys.modules['antenv.axon_hooks'] = mod
    antenv.axon_hooks = mod
    try:
        lib = ctypes.CDLL('/opt/axon/libaxon_pjrt.so')
    except OSError:
        return
    if not hasattr(lib, 'axon_start_nrt_profile'):
        return
    lib.axon_start_nrt_profile.argtypes = [ctypes.POINTER(ctypes.c_int64),
                                           ctypes.c_size_t]
    lib.axon_start_nrt_profile.restype = ctypes.c_int64
    lib.axon_stop_nrt_profile.argtypes = [ctypes.c_char_p]
    lib.axon_stop_nrt_profile.restype = ctypes.c_int64

    @contextlib.contextmanager
    def _hook(output_dir, device_ids):
        import jax
        jax.devices()
        if device_ids:
            ids = (ctypes.c_int64 * len(device_ids))(*device_ids)
            rc = lib.axon_start_nrt_profile(ids, len(device_ids))
        else:
            rc = lib.axon_start_nrt_profile(None, 0)
        if rc != 0:
            raise RuntimeError(f"axon_start_nrt_profile rc={rc}")
        try:
            yield
        finally:
            n = lib.axon_stop_nrt_profile(str(output_dir).encode())
            print(f"profile: {n} ntff file(s) in {output_dir}", file=sys.stderr)

    mod.set_axon_ntff_profile_hook = mod.set_axon_ntff_profile_hook
    mod.set_axon_ntff_profile_hook(_hook)
    import concourse.bass_utils as bu
    bu.upload_artifacts = lambda tmpdir: "local://" + str(tmpdir)


# revision 108
# speedup vs baseline: 1.1944x; 1.0435x over previous
"""Trainium2 Bass kernel for nn_AttentionModelCharLevel.

Model (per reference): visual linear -> char-encoder LSTM -> linear+relu ->
cosine attention (softmax over batch dim) -> char-decoder LSTM -> per-sample
mean NLL over L steps.

Sharding: data-parallel over batch B=4096 across 8 cores (512 rows each).
The [B,B] attention needs every core to see all normalized visual rows, so
each core computes its vn shard (+transpose), AllGathers both, and streams
the gathered rows back through SBUF during the attention phase.

Key device-side conventions (v2, fp8 DoubleRow):
- The LSTM recurrence, encoder linear and decoder logits matmuls run in
  fp8e4 with perf_mode=DoubleRow (2 fp8 K-rows per PE cell): K=512
  contractions become 2 matmuls of logical K=256. Weights are pre-scaled
  x64 host-side (fp8 normal range) and the x1/64 is folded into the ACT
  scale that reads the PSUM.
- Hidden state is stored doubled (Ht = 2h) as a single [128, 4, B] fp8
  tile per step; slice [:, 2g:2g+2, :] is the DoubleRow rhs pair. Cell
  state St = 2c is a [128, 4, B] bf16 tile.
- sigmoid(z) = 0.5*tanh(z/2)+0.5; the g-gate's weights carry an extra x2
  so ALL gates share one ACT scale (0.5/64). The bias is folded into the
  x-part matmul as a 51st embedding row (x=1), so one ACT with no bias
  covers a whole 4-bank PSUM quad [128, 4, B] = the 4 gates of one hidden
  chunk -> 4 gate ACTs + 1 tanh(c) ACT per step instead of 20.
- The DVE tail chain per hidden chunk runs in bf16 (2x DVE rate):
      m2 = (tanh_i + 1) * tanh_g
      m1 = (tanh_f + 1) * S
      S' = 0.5*m1 + m2
      H' = (tanh_o + 1) * tanh(0.5*S')   (fp8 out)
- Softmax over the batch dim reduces to exp() only: sims are cosine
  similarities in [-1,1] and the softmax denominator is a positive
  per-column scale that the final row normalization of h cancels.
- Decoder log-softmax: logits kept in [V, B] layout; Z = ones^T exp(logits)
  and the target logit via a one-hot mask built from an iota column --
  both reduce over partitions with K=128 ones-matmuls into spare banks of
  the rotating PSUM quads. ln() deferred to a single post-pass.
"""
import os
import sys

sys.path.insert(0, '/opt/trn_rl_repo')

import numpy as np

B_FULL = 4096
NCORES = 8
B = B_FULL // NCORES          # 512 rows per core
H = 512
G = 4 * H                     # 2048
E = 50
EA = E + 1                    # embedding dim + bias row
L = 16
V = 128
VIS = 2048
HK = H // 128                 # 4 chunks of the hidden dim
GK = G // 128                 # 16 gate chunks
VISK = VIS // 128             # 16 chunks of the visual dim
BK = B // 128                 # 4 batch chunks per core
VCHUNKS = B_FULL // 128       # 32 chunks of the full batch
SF = 64.0                     # fp8 weight scale

_CACHE = {}


def _build():
    import concourse.bass as bass
    import concourse.tile as tile
    import concourse.mybir as mybir
    from concourse import bacc
    from concourse.masks import make_identity
    from contextlib import ExitStack

    dt = mybir.dt
    AF = mybir.ActivationFunctionType
    ALU = mybir.AluOpType
    DR = mybir.MatmulPerfMode.DoubleRow
    f32 = dt.float32
    f32r = dt.float32r
    bf16d = dt.bfloat16
    f8 = dt.float8e4
    LN2 = float(np.log(2.0))

    AP = bass.AP
    nc = bacc.Bacc("TRN2", target_bir_lowering=False, debug=False,
                   num_devices=NCORES)

    # ---- DRAM I/O ----
    visT_d = nc.dram_tensor("visT", [VISK // 2, 128, 2, B], f8, kind="ExternalInput").ap()
    WvisT_d = nc.dram_tensor("WvisT", [VISK // 2, 128, 2, H], f8, kind="ExternalInput").ap()
    WihT_d = nc.dram_tensor("WihT", [EA, G], bf16d, kind="ExternalInput").ap()
    Whh8_d = nc.dram_tensor("Whh8", [128, HK, G], f8, kind="ExternalInput").ap()
    encx_d = nc.dram_tensor("encx", [L, EA, B], bf16d, kind="ExternalInput").ap()
    decx_d = nc.dram_tensor("decx", [L, EA, B], bf16d, kind="ExternalInput").ap()
    Wenc8_d = nc.dram_tensor("Wenc8", [128, HK, H], f8, kind="ExternalInput").ap()
    benc_d = nc.dram_tensor("benc", [128, HK], f32, kind="ExternalInput").ap()
    Wout8_d = nc.dram_tensor("Wout8", [128, HK, V], f8, kind="ExternalInput").ap()
    tgt_d = nc.dram_tensor("tgt", [L, B], f32, kind="ExternalInput").ap()
    iota_d = nc.dram_tensor("iota128", [128, 1], f32, kind="ExternalInput").ap()
    oneh_d = nc.dram_tensor("oneh", [L, 128, B], f8, kind="ExternalInput").ap()
    ones_d = nc.dram_tensor("ones128", [128, 1], f32r, kind="ExternalInput").ap()
    out_d = nc.dram_tensor("loss", [1, B], f32, kind="ExternalOutput").ap()

    with tile.TileContext(nc) as tc, ExitStack() as top:
        wpool = top.enter_context(tc.tile_pool(name="w", bufs=1))
        spool = top.enter_context(tc.tile_pool(name="state", bufs=2))
        dram = top.enter_context(tc.tile_pool(name="dram", bufs=1, space="DRAM"))

        # ---- persistent weights / constants (DMAs issued after the visual
        # input DMAs: the scan weights aren't needed until the encoder) ----
        Whh8 = wpool.tile([128, HK, G], f8, tag="whh8", name="whh8")
        WihT2 = wpool.tile([64 + EA, G], bf16d, tag="wih", name="wih")
        Wenc8 = wpool.tile([128, HK, H], f8, tag="wenc8", name="wenc8")
        benc = wpool.tile([128, HK], f32, tag="benc", name="benc")
        Wout8 = wpool.tile([128, HK, V], f8, tag="wout8", name="wout8")

        def load_weights():
            nc.sync.dma_start(Whh8[:], Whh8_d)
            nc.sync.dma_start(WihT2[:EA, :], WihT_d)
            nc.sync.dma_start(WihT2[64:64 + EA, :], WihT_d)
            nc.sync.dma_start(Wenc8[:], Wenc8_d)
            nc.sync.dma_start(benc[:], benc_d)
            nc.sync.dma_start(Wout8[:], Wout8_d)

        ones_col = wpool.tile([128, 1], f32r, tag="ones_col", name="ones_col")
        nc.sync.dma_start(ones_col[:], ones_d)
        ones_row = wpool.tile([1, 128], f32r, tag="ones_row", name="ones_row")
        nc.sync.dma_start(ones_row[:], ones_d.rearrange("p one -> one p"))
        ones16 = wpool.tile([L, 1], f32r, tag="ones16", name="ones16")
        nc.sync.dma_start(ones16[:], ones_d[:L])
        ident8 = wpool.tile([128, 128], f8, tag="ident8", name="ident8")
        make_identity(nc, ident8[:])

        def warm_pe(out_ap, n):
            # dummy matmuls into a region the next real start=True matmul
            # resets; they run in otherwise-idle PE windows and keep the
            # HAM activity window busy so real matmuls stay at 2.4 GHz
            for _ in range(n):
                nc.tensor.matmul(out_ap, ident8[:], ident8[:],
                                 start=True, stop=True,
                                 skip_group_check=True)
        ln2_t = wpool.tile([1, 1], f32, tag="ln2", name="ln2")
        nc.vector.memset(ln2_t[:], LN2)
        ln16_t = wpool.tile([1, 1], f32, tag="ln16", name="ln16")
        nc.vector.memset(ln16_t[:], float(np.log(16.0)))
        ln16c = wpool.tile([128, 1], f32, tag="ln16c", name="ln16c")
        nc.vector.memset(ln16c[:], float(np.log(16.0)))

        # decoder per-step Z and target-logit rows (opened before vload so
        # vload can close LIFO after phase 1)
        zpool = top.enter_context(tc.tile_pool(name="zp", bufs=1))
        Zboth = zpool.tile([L, 2, B], f32r, tag="Zb", name="Zb")
        Zs = Zboth[:, 0, :]
        lts = Zboth[:, 1, :]

        # visual inputs land first, split across BOTH DMA queues so the
        # visual matmuls aren't paced by one queue's dispatch rate; the
        # pool closes after phase 1 to return the SBUF (LIFO: innermost
        # of the currently-open pools at that point)
        vload_stack = ExitStack()
        vload = vload_stack.enter_context(tc.tile_pool(name="vload", bufs=1))
        vis_ts = [vload.tile([128, 2, B], f8, tag=f"vis{k}", name=f"vis{k}")
                  for k in range(VISK // 2)]
        wv_ts = [vload.tile([128, 2, H], f8, tag=f"wvis{k}", name=f"wvis{k}")
                 for k in range(VISK // 2)]
        for ki in range(VISK // 2):
            nc.gpsimd.dma_start(vis_ts[ki][:], visT_d[ki])
            nc.sync.dma_start(wv_ts[ki][:], WvisT_d[ki])

        # prefetch ALL scan inputs up front on the gpsimd DMA queue so the
        # LSTM scans never wait on the sync queue / collective window
        xenc = wpool.tile([64 + EA, L, B], bf16d, tag="xenc", name="xenc")
        xdec = wpool.tile([64 + EA, L, B], bf16d, tag="xdec", name="xdec")
        for s in range(L):
            nc.gpsimd.dma_start(xenc[:EA, s, :], encx_d[s])
            nc.gpsimd.dma_start(xenc[64:64 + EA, s, :], encx_d[s])
        for s in range(L):
            nc.gpsimd.dma_start(xdec[:EA, s, :], decx_d[s])
            nc.gpsimd.dma_start(xdec[64:64 + EA, s, :], decx_d[s])
        ohall = wpool.tile([128, L, B], f8, tag="ohall", name="ohall")
        nc.gpsimd.dma_start(ohall[:], oneh_d.rearrange("l p b -> p l b"))

        # AllGather buffers: vn blocks and vnT blocks (both fp8, x16 scale)
        ag_in = dram.tile([BK, 128, B], f8, name="ag_in")
        ag_out = dram.tile([NCORES, BK, 128, B], f8, addr_space="Shared", name="ag_out")
        agt_in = dram.tile([HK, 128, B], f8, name="agt_in")
        agt_out = dram.tile([NCORES, HK, 128, B], f8, addr_space="Shared", name="agt_out")



        # ======== Phase 1: visual linear + row-normalize + transpose + AG ====
        with ExitStack() as ph:
            vsb = ph.enter_context(tc.tile_pool(name="vsb", bufs=3))
            vps = ph.enter_context(tc.tile_pool(name="vps", bufs=1, space="PSUM"))
            tps = ph.enter_context(tc.tile_pool(name="tps", bufs=4, space="PSUM"))
            vnpool = ph.enter_context(tc.tile_pool(name="vnp", bufs=1))

            v_ps = [vps.tile([128, H], f32, tag=f"vps{b}", name=f"vps{b}") for b in range(BK)]
            for ki in range(VISK // 2):
                vis_t, wv_t = vis_ts[ki], wv_ts[ki]
                for b in range(BK):
                    nc.tensor.matmul(v_ps[b][:], vis_t[:, :, b * 128:(b + 1) * 128],
                                     wv_t[:], start=(ki == 0),
                                     stop=(ki == VISK // 2 - 1), perf_mode=DR)
            load_weights()
            # batched by function so the ACT table isn't thrashed
            s_cols, vn = [], []
            for b in range(BK):
                sq = vsb.tile([128, H], f32, tag="vsq", name="vsq")
                s_col = vsb.tile([128, 1], f32, tag=f"vscol{b}", name=f"vscol{b}", bufs=1)
                nc.scalar.activation(sq[:], v_ps[b][:], AF.Square,
                                     accum_out=s_col[:])
                s_cols.append(s_col)
            lns = []
            for b in range(BK):
                lnc_ = vsb.tile([128, 1], f32, tag=f"vln{b}", name=f"vln{b}", bufs=1)
                nc.scalar.activation(lnc_[:], s_cols[b][:], AF.Ln)
                lns.append(lnc_)
            # sqrt(s) = 64*|v|, so exp(-0.5*ln s + ln16) puts vn in fp8 x16
            rss = []
            for b in range(BK):
                rs = vsb.tile([128, 1], f32, tag=f"vrs{b}", name=f"vrs{b}", bufs=1)
                nc.scalar.activation(rs[:], lns[b][:], AF.Exp, scale=-0.5,
                                     bias=ln16c[:])
                rss.append(rs)
            for b in range(BK):
                vn_b = vnpool.tile([128, H], f8, tag=f"vn{b}", name=f"vn{b}")
                nc.vector.tensor_scalar(vn_b[:], v_ps[b][:], rss[b][:], None, ALU.mult)
                vn.append(vn_b)
            # transpose vn -> vnT (16 128x128 blocks, fp8 for the sims lhsT)
            vnT = [vnpool.tile([128, B], f8, tag=f"vnT{h}", name=f"vnT{h}") for h in range(HK)]
            for b in range(BK):
                for h in range(HK):
                    t_ps = tps.tile([128, 128, 2], f8, tag="tr", name="tr")
                    nc.tensor.transpose(
                        t_ps[:, :, 0], vn[b][:, h * 128:(h + 1) * 128], ident8[:])
                    nc.vector.tensor_copy(vnT[h][:, b * 128:(b + 1) * 128],
                                          t_ps[:, :, 0])
            for b in range(BK):
                nc.sync.dma_start(ag_in[b], vn[b][:])
            for h in range(HK):
                nc.sync.dma_start(agt_in[h], vnT[h][:])
            nc.gpsimd.collective_compute(
                "AllGather", mybir.AluOpType.bypass,
                replica_groups=[list(range(NCORES))],
                ins=[ag_in[:]], outs=[ag_out[:]],
            )
            nc.gpsimd.collective_compute(
                "AllGather", mybir.AluOpType.bypass,
                replica_groups=[list(range(NCORES))],
                ins=[agt_in[:]], outs=[agt_out[:]],
            )

        vload_stack.close()
        # staged full vnT (fp8, DoubleRow pair layout) for the sims lhsT;
        # filled by gpsimd DMAs that wait on the collective during the encoder
        vnTp = [wpool.tile([128, 2, B_FULL], f8, tag=f"vnTp{g}", name=f"vnTp{g}")
                for g in range(2)]
        for k in range(HK):
            for r in range(NCORES):
                nc.gpsimd.dma_start(vnTp[k // 2][:, k % 2, r * B:(r + 1) * B],
                                    agt_out[r, k])

        # ======== LSTM scan helper ========
        gsb = top.enter_context(tc.tile_pool(name="gsb", bufs=3))
        msb = top.enter_context(tc.tile_pool(name="msb", bufs=4))

        from collections import deque

        def lstm_step(gps, xall, s, Hp8, Sp, max_open=2):
            """One LSTM step, fp8 DoubleRow recurrence, quad-bank PSUM.

            Per hidden chunk j one 4-bank PSUM quad holds gates i,f,g,o.
            open = x-part matmuls (K=51, two concurrent row groups) plus the
            h01 DoubleRow pair; close = h23 pair + one mega-ACT over the quad
            + the state chain split across gpsimd (m2, m1) and DVE (S', H'),
            with tanh(c) in chunk pairs on ACT. Closes lag opens so the PE
            streams the next chunk while the previous drains, and the first
            closed chunks feed the next step's opens.
            """
            Hn8 = spool.tile([128, HK, B], f8, tag="H", name="H")
            Sn = spool.tile([128, HK, B], bf16d, tag="S", name="S")
            Ts = {}
            pairs = {}

            def xmms(j, half):
                pt = gps.tile([128, 2, B], f32, tag="pair", name="pair")
                pairs[(j, half)] = pt
                for gi in range(2):
                    gate = half * 2 + gi
                    c = gate * 4 + j
                    r0 = 0 if gi == 0 else 64
                    nc.tensor.matmul(pt[:, gi, :],
                                     WihT2[r0:r0 + EA, c * 128:(c + 1) * 128],
                                     xall[r0:r0 + EA, s, :], start=True, stop=False)

            def dr(j, half, lo, hi):
                pt = pairs[(j, half)]
                for gi in range(2):
                    gate = half * 2 + gi
                    c = gate * 4 + j
                    nc.tensor.matmul(pt[:, gi, :],
                                     Whh8[:, lo:hi, c * 128:(c + 1) * 128],
                                     Hp8[:, lo:hi, :], start=False,
                                     stop=(hi == HK), perf_mode=DR)

            def acts(j):
                # T layout [128, gate, chunk%2, B]: each gate's two chunks
                # are contiguous, so the whole chunk-pair tail chain runs as
                # four [128, 2, B] DVE ops instead of eight chunk ops
                T = Ts[j // 2 * 2]
                if T is None:
                    T = gsb.tile([128, 4, 2, B], bf16d, tag="T", name="T")
                    Ts[j // 2 * 2] = T
                c = j % 2
                nc.scalar.activation(T[:, 0:2, c, :], pairs[(j, 0)][:],
                                     AF.Tanh, scale=0.5 / SF)
                nc.scalar.activation(T[:, 2:4, c, :], pairs[(j, 1)][:],
                                     AF.Tanh, scale=0.5 / SF)

            def chainP(jlo):  # m1/m2/S' per chunk (short serial latency)
                T = Ts[jlo]
                for c in (0, 1):
                    j = jlo + c
                    m1 = msb.tile([128, B], bf16d, tag="m1", name="m1")
                    nc.vector.scalar_tensor_tensor(m1[:], T[:, 1, c, :], 1.0,
                                                   Sp[:, j, :],
                                                   ALU.add, ALU.mult)
                    m2 = msb.tile([128, B], bf16d, tag="m2", name="m2")
                    nc.vector.scalar_tensor_tensor(m2[:], T[:, 0, c, :], 1.0,
                                                   T[:, 2, c, :],
                                                   ALU.add, ALU.mult)
                    nc.vector.scalar_tensor_tensor(Sn[:, j, :], m1[:],
                                                   0.5, m2[:],
                                                   ALU.mult, ALU.add)

            def thH(jlo):  # tanh(c) for the chunk pair, then per-chunk H'
                th = msb.tile([128, 2, B], bf16d, tag="th", name="th")
                nc.scalar.activation(th[:], Sn[:, jlo:jlo + 2, :],
                                     AF.Tanh, scale=0.5)
                for c in (0, 1):
                    nc.vector.scalar_tensor_tensor(Hn8[:, jlo + c, :],
                                                   Ts[jlo][:, 3, c, :], 1.0,
                                                   th[:, c, :], ALU.add, ALU.mult)

            Ts = {0: None, 2: None}
            # 2-bank pair tiles, 4 in flight: two chunks of PE runway so the
            # ACT read latency never gaps the PE (keeps HAM warm). x-matmuls
            # of chunks 0/1 are H-independent and absorb the previous step's
            # tail; the tail chain runs at chunk-pair granularity and lags
            # so it never head-of-line blocks the gate ACTs.
            xmms(0, 0); xmms(0, 1); xmms(1, 0); xmms(1, 1)
            dr(0, 0, 0, 2); dr(0, 0, 2, 4); dr(0, 1, 0, 2); dr(0, 1, 2, 4)
            acts(0)
            dr(1, 0, 0, 2); dr(1, 0, 2, 4); dr(1, 1, 0, 2); dr(1, 1, 2, 4)
            acts(1)
            xmms(2, 0); xmms(2, 1)
            dr(2, 0, 0, 2); dr(2, 0, 2, 4); dr(2, 1, 0, 2); dr(2, 1, 2, 4)
            acts(2)
            chainP(0)
            thH(0)
            xmms(3, 0); xmms(3, 1)
            dr(3, 0, 0, 2); dr(3, 0, 2, 4); dr(3, 1, 0, 2); dr(3, 1, 2, 4)
            acts(3)
            chainP(2)
            # tail-critical pair: per-chunk th so H'_2 / H'_3 land earlier
            for j in (2, 3):
                th1 = msb.tile([128, B], bf16d, tag="th1", name="th1")
                nc.scalar.activation(th1[:], Sn[:, j, :], AF.Tanh, scale=0.5)
                nc.vector.scalar_tensor_tensor(Hn8[:, j, :],
                                               Ts[2][:, 3, j - 2, :], 1.0,
                                               th1[:], ALU.add, ALU.mult)
            return Hn8, Sn

        # ======== Phase 2: encoder ========
        Hp8 = spool.tile([128, HK, B], f8, tag="H", name="H")
        Sp = spool.tile([128, HK, B], bf16d, tag="S", name="S")
        nc.vector.memset(Hp8[:], 0.2)
        nc.vector.memset(Sp[:], 0.2)
        with tc.tile_pool(name="gpse", bufs=4, space="PSUM") as gps_e:
            for s in range(L):
                Hp8, Sp = lstm_step(gps_e, xenc, s, Hp8, Sp)
        Henc = Hp8

        # ======== Phase 3: t path + attention ========
        with ExitStack() as ph:
            asb = ph.enter_context(tc.tile_pool(name="asb", bufs=2))
            vstr = ph.enter_context(tc.tile_pool(name="vstr", bufs=3))

            # --- 3a: t = relu(Wenc' @ Henc + benc), column-normalized ---
            # own PSUM scope so the attention loop below gets the banks back
            ph3a = ph.enter_context(ExitStack())
            aps = ph3a.enter_context(tc.tile_pool(name="aps", bufs=1, space="PSUM"))
            sps_pool = ph3a.enter_context(tc.tile_pool(name="sps", bufs=2, space="PSUM"))
            tra = asb.tile([128, HK, B], f32, tag="tra", name="tra", bufs=1)
            sqa = asb.tile([128, HK, B], f32r, tag="tsqa", name="tsqa", bufs=1)
            s_ps = aps.tile([1, B], f32, tag="tsum", name="tsum")
            bc_ps = aps.tile([128, B], f32, tag="tbc", name="tbc")
            for mi in range(HK):
                t_ps = sps_pool.tile([128, B], f32, tag="sims", name="sims")
                nc.tensor.matmul(t_ps[:], Wenc8[:, 0:2, mi * 128:(mi + 1) * 128],
                                 Henc[:, 0:2, :], start=True, stop=False,
                                 perf_mode=DR)
                nc.tensor.matmul(t_ps[:], Wenc8[:, 2:4, mi * 128:(mi + 1) * 128],
                                 Henc[:, 2:4, :], start=False, stop=True,
                                 perf_mode=DR)
                nc.scalar.activation(tra[:, mi, :], t_ps[:], AF.Relu,
                                     scale=1.0 / SF, bias=benc[:, mi:mi + 1])
            pass  # warm_pe removed
            nc.scalar.activation(sqa[:], tra[:], AF.Square)
            for mi in range(HK):
                nc.tensor.matmul(s_ps[:], ones_col[:], sqa[:, mi, :],
                                 start=(mi == 0), stop=(mi == HK - 1))
            lnr = asb.tile([1, B], f32, tag="tlnr", name="tlnr")
            nc.scalar.activation(lnr[:], s_ps[:], AF.Ln)
            rs_r = asb.tile([1, B], f32r, tag="trs", name="trs")
            nc.scalar.activation(rs_r[:], lnr[:], AF.Exp, scale=-0.5,
                                 bias=ln16_t[:])
            nc.tensor.matmul(bc_ps[:], ones_row[:], rs_r[:], start=True, stop=True)
            tnp = [asb.tile([128, 2, B], f8, tag=f"tnp{g}", name=f"tnp{g}", bufs=1)
                   for g in range(2)]
            for mi in range(HK):
                nc.vector.tensor_tensor(tnp[mi // 2][:, mi % 2, :],
                                        tra[:, mi, :], bc_ps[:], ALU.mult)
            ph3a.close()

            # --- 3b: attention. Stream gathered vn pairs, E = 16*exp(sims)
            # in fp8, accumulate h with DoubleRow pairs over batch chunks.
            # 4 sim buffers keep 4 blocks in flight so the PE stays dense.
            hps_pool = ph.enter_context(tc.tile_pool(name="hps", bufs=1, space="PSUM"))
            sps_pool = ph.enter_context(tc.tile_pool(name="sps2", bufs=4, space="PSUM"))
            hua = hps_pool.tile([128, HK, B], f32, tag="hua", name="hua")
            hu_ps = [hua[:, h, :] for h in range(HK)]
            pass  # warm_pe removed
            for ip in range(VCHUNKS // 2):
                vnp_i = vstr.tile([128, 2, H], f8, tag="vni", name="vni", bufs=8)
                Ep = vstr.tile([128, 2, B], f8, tag="E", name="E", bufs=6)
                for t_ in range(2):
                    i = 2 * ip + t_
                    r, b = divmod(i, BK)
                    nc.sync.dma_start(vnp_i[:, t_, :], ag_out[r, b])
                    sim_ps = sps_pool.tile([128, B], f32, tag="sims", name="sims")
                    for g in range(2):
                        nc.tensor.matmul(sim_ps[:],
                                         vnTp[g][:, :, i * 128:(i + 1) * 128],
                                         tnp[g][:], start=(g == 0),
                                         stop=(g == 1), perf_mode=DR)
                    nc.scalar.activation(Ep[:, t_, :], sim_ps[:], AF.Exp,
                                         scale=1.0 / 256, bias=ln16c[:])
                for h in range(HK):
                    nc.tensor.matmul(hu_ps[h],
                                     vnp_i[:, :, h * 128:(h + 1) * 128], Ep[:],
                                     start=(ip == 0),
                                     stop=(ip == VCHUNKS // 2 - 1),
                                     perf_mode=DR)
            # normalize h (x2 for the doubled-state convention) -> decoder init
            s2_full = sps_pool.tile([128, B], f32, tag="sims", name="sims")
            s2_ps = s2_full[0:1, :]
            pass  # warm_pe removed
            squ = asb.tile([128, HK, B], f32r, tag="husq", name="husq")
            nc.scalar.activation(squ[:], hua[:], AF.Square)
            for h in range(HK):
                nc.tensor.matmul(s2_ps, ones_col[:], squ[:, h, :],
                                 start=(h == 0), stop=(h == HK - 1))
            lnr2 = asb.tile([1, B], f32, tag="hulnr", name="hulnr")
            nc.scalar.activation(lnr2[:], s2_ps, AF.Ln)
            rs2 = asb.tile([1, B], f32r, tag="hurs", name="hurs")
            nc.scalar.activation(rs2[:], lnr2[:], AF.Exp, scale=-0.5, bias=ln2_t[:])
            bc2_full = sps_pool.tile([128, B], f32, tag="sims", name="sims")
            nc.tensor.matmul(bc2_full[:], ones_row[:], rs2[:], start=True, stop=True)
            bc2_sb = asb.tile([128, B], f32, tag="bc2sb", name="bc2sb", bufs=1)
            nc.vector.tensor_copy(bc2_sb[:], bc2_full[:])
            # decoder init: c0 = h0, so H0 doubles as the initial cell state
            # (step 0's m1 reads it as fp8 -- validated headroom)
            H0 = spool.tile([128, HK, B], f8, tag="H", name="H")
            for h in range(HK):
                nc.vector.tensor_tensor(H0[:, h, :], hu_ps[h], bc2_sb[:], ALU.mult)

        # ======== Phase 4: decoder ========
        dsb = top.enter_context(tc.tile_pool(name="dsb", bufs=2))
        with tc.tile_pool(name="gpsd", bufs=4, space="PSUM") as gps_d:
            pass  # warm_pe removed
            Hp8, Sp = H0, H0
            for s in range(L):
                Hp8, Sp = lstm_step(gps_d, xdec, s, Hp8, Sp)
                # logitsT [V, B] in bank 0 of a rotating pair tile; Z and
                # target exp(logit) reductions land in bank 1 (partitions 0/32)
                lq = gps_d.tile([128, 2, B], f32, tag="pair", name="pair")
                nc.tensor.matmul(lq[:, 0, :], Wout8[:, 0:2, :], Hp8[:, 0:2, :],
                                 start=True, stop=False, perf_mode=DR)
                nc.tensor.matmul(lq[:, 0, :], Wout8[:, 2:4, :], Hp8[:, 2:4, :],
                                 start=False, stop=True, perf_mode=DR)
                El = dsb.tile([128, B], f32r, tag="El", name="El")
                nc.scalar.activation(El[:], lq[:, 0, :], AF.Exp, scale=1.0 / SF)
                nc.tensor.matmul(lq[0:1, 1, :], ones_col[:], El[:],
                                 start=True, stop=True)
                ztmp = dsb.tile([1, B], f32r, tag="ztmp", name="ztmp")
                nc.vector.tensor_copy(ztmp[:], lq[0:1, 1, :])
                nc.sync.dma_start(Zboth[s:s + 1, 0, :], ztmp[:])
                # exp(target logit) via host one-hot * El (on the otherwise
                # idle gpsimd); ln() undoes the exp in the post-pass
                mk = dsb.tile([128, B], f32r, tag="mk", name="mk")
                nc.gpsimd.tensor_tensor(mk[:], ohall[:, s, :], El[:], ALU.mult)
                nc.tensor.matmul(lq[0:1, 1, :], ones_col[:], mk[:],
                                 start=True, stop=True)
                lttmp = dsb.tile([1, B], f32r, tag="lttmp", name="lttmp")
                nc.vector.tensor_copy(lttmp[:], lq[0:1, 1, :])
                nc.sync.dma_start(Zboth[s:s + 1, 1, :], lttmp[:])

            # ======== Phase 5: final loss ========
            lnB = dsb.tile([L, 2, B], f32r, tag="lnB", name="lnB")
            nc.scalar.activation(lnB[:], Zboth[:], AF.Ln)
            diff = dsb.tile([L, B], f32r, tag="diff", name="diff")
            nc.vector.tensor_tensor(diff[:], lnB[:, 0, :], lnB[:, 1, :],
                                    ALU.subtract)
            fq = gps_d.tile([128, 2, B], f32, tag="pair", name="pair")
            nc.tensor.matmul(fq[0:1, 0, :], ones16[:], diff[:], start=True, stop=True)
            loss_sb = dsb.tile([1, B], f32, tag="losssb", name="losssb")
            nc.vector.tensor_scalar(loss_sb[:], fq[0:1, 0, :], 1.0 / L, None,
                                    ALU.mult)
            nc.sync.dma_start(out_d, loss_sb[:])

    nc.compile()
    return nc


def _prep_inputs(visual_input, text_input, emb, W_ih, W_hh, b_ih, b_hh,
                 W_enc, b_enc, W_out, W_vis):
    import ml_dtypes
    bf = ml_dtypes.bfloat16
    f8n = ml_dtypes.float8_e4m3
    f = np.float32
    vis = np.asarray(visual_input, f)[:, 0, :]              # [4096, 2048]
    text = np.asarray(text_input)
    emb = np.asarray(emb, f)
    visT = np.ascontiguousarray(vis.T)                      # [2048, 4096]
    WvisT = (SF * np.asarray(W_vis, f)).T                   # [2048, 512]
    Wvis8 = np.ascontiguousarray(
        WvisT.reshape(VISK // 2, 2, 128, H).transpose(0, 2, 1, 3)).astype(f8n)

    # x-part weights x64 with bias folded as row 50; g-gate block x2 so all
    # gates share the ACT scale 0.5/64
    b = np.asarray(b_ih, f) + np.asarray(b_hh, f)           # [2048]
    WihT = np.concatenate([np.asarray(W_ih, f).T, b[None, :]], axis=0) * SF
    WihT[:, 2 * H:3 * H] *= 2.0                             # [51, 2048]

    def pack8(WT):                                          # [512, M] -> [128, 4, M]
        M = WT.shape[1]
        return np.ascontiguousarray(
            WT.reshape(HK, 128, M).transpose(1, 0, 2)).astype(f8n)

    WhhT = (0.5 * SF) * np.asarray(W_hh, f).T               # [512, 2048]
    WhhT[:, 2 * H:3 * H] *= 2.0
    Whh8 = pack8(WhhT)
    Wenc8 = pack8((0.5 * SF) * np.asarray(W_enc, f).T)      # [512, 512]
    Wout8 = pack8((0.5 * SF) * np.asarray(W_out, f).T)      # [512, 128]
    benc = np.ascontiguousarray(np.asarray(b_enc, f).reshape(HK, 128).T)

    encx = emb[text.T]                                      # [16, 4096, 50]
    dec_ch = np.concatenate([np.zeros((text.shape[0], 1), text.dtype),
                             text[:, :-1]], axis=1)
    decx = emb[dec_ch.T]                                    # [16, 4096, 50]
    one_row = np.ones((L, 1, B_FULL), f)
    encxT = np.concatenate([encx.transpose(0, 2, 1), one_row], axis=1)
    decxT = np.concatenate([decx.transpose(0, 2, 1), one_row], axis=1)
    encxT = np.ascontiguousarray(encxT)                     # [16, 51, 4096]
    decxT = np.ascontiguousarray(decxT)
    tgt = np.ascontiguousarray(text.T.astype(f))            # [16, 4096]
    iota = np.arange(128, dtype=f).reshape(128, 1)
    # one-hot target masks [L, V, B_FULL]
    oneh = (tgt[:, None, :] == iota.reshape(1, 128, 1)).astype(f)

    in_maps = []
    for c in range(NCORES):
        sl = slice(c * B, (c + 1) * B)
        in_maps.append({
            "visT": np.ascontiguousarray(
                visT[:, sl].reshape(VISK // 2, 2, 128, B).transpose(0, 2, 1, 3)
            ).astype(f8n),
            "WvisT": Wvis8,
            "WihT": WihT.astype(bf),
            "Whh8": Whh8,
            "encx": np.ascontiguousarray(encxT[:, :, sl]).astype(bf),
            "decx": np.ascontiguousarray(decxT[:, :, sl]).astype(bf),
            "Wenc8": Wenc8,
            "benc": benc,
            "Wout8": Wout8,
            "tgt": np.ascontiguousarray(tgt[:, sl]),
            "iota128": iota,
            "oneh": np.ascontiguousarray(oneh[:, :, sl]).astype(f8n),
            "ones128": np.ones((128, 1), np.float32),
        })
    return in_maps


LAST_EXEC_TIME_NS = None


def kernel(**inputs):
    global LAST_EXEC_TIME_NS
    from concourse.bass_utils import run_bass_kernel_spmd

    if "nc" not in _CACHE:
        _CACHE["nc"] = _build()
    nc = _CACHE["nc"]
    in_maps = _prep_inputs(**inputs)

    trace = bool(int(os.environ.get("KERNEL_PROFILE", "0")))
    kw = {}
    if trace:
        _install_profile_hook()
        kw["trace"] = True
    res = run_bass_kernel_spmd(nc, in_maps, core_ids=list(range(NCORES)), **kw)
    LAST_EXEC_TIME_NS = res.exec_time_ns
    out = np.concatenate([res.results[c]["loss"][0] for c in range(NCORES)])
    return out.astype(np.float32)


def _install_profile_hook():
    """Optional NTFF profiling (dev only; used when KERNEL_PROFILE=1)."""
    import types, ctypes, contextlib
    try:
        import antenv
    except ImportError:
        return
    if getattr(antenv, "axon_hooks", None) is not None:
        return
    mod = types.ModuleType('antenv.axon_hooks')
    _store = [None]
    mod.set_axon_ntff_profile_hook = lambda h: _store.__setitem__(0, h)
    mod.get_axon_ntff_profile_hook = lambda: _store[0]
    sys.modules['antenv.axon_hooks'] = mod
    antenv.axon_hooks = mod
    try:
        lib = ctypes.CDLL('/opt/axon/libaxon_pjrt.so')
    except OSError:
        return
    if not hasattr(lib, 'axon_start_nrt_profile'):
        return
    lib.axon_start_nrt_profile.argtypes = [ctypes.POINTER(ctypes.c_int64),
                                           ctypes.c_size_t]
    lib.axon_start_nrt_profile.restype = ctypes.c_int64
    lib.axon_stop_nrt_profile.argtypes = [ctypes.c_char_p]
    lib.axon_stop_nrt_profile.restype = ctypes.c_int64

    @contextlib.contextmanager
    def _hook(output_dir, device_ids):
        import jax
        jax.devices()
        if device_ids:
            ids = (ctypes.c_int64 * len(device_ids))(*device_ids)
            rc = lib.axon_start_nrt_profile(ids, len(device_ids))
        else:
            rc = lib.axon_start_nrt_profile(None, 0)
        if rc != 0:
            raise RuntimeError(f"axon_start_nrt_profile rc={rc}")
        try:
            yield
        finally:
            n = lib.axon_stop_nrt_profile(str(output_dir).encode())
            print(f"profile: {n} ntff file(s) in {output_dir}", file=sys.stderr)

    mod.set_axon_ntff_profile_hook = mod.set_axon_ntff_profile_hook
    mod.set_axon_ntff_profile_hook(_hook)
    import concourse.bass_utils as bu
    bu.upload_artifacts = lambda tmpdir: "local://" + str(tmpdir)
